# revision 10
# baseline (speedup 1.0000x reference)
"""Trainium2 Bass kernel for nn_MultiHeadAttention_55894704390646.

Multi-head causal attention, B=2, S=2048, E=1024, H=16 heads, D=64.
Sharding: data-parallel over batch (2 groups) x tensor-parallel over heads
(4 heads per core). Each core computes a partial output-projection result
(row-split Wo); the host sums the 4 partials per batch and adds the bias.

v2 design (all matmul operands bf16, f32 PSUM accumulation):
  - host supplies x^T [E, S] and pre-transposed weight slices in bf16, so
    every matmul contraction dim lands on SBUF partitions with no on-device
    transposes, and input DMA bytes are halved.
  - scores computed transposed ([keys, queries]); softmax uses exp(s/8)
    (no max subtraction: |s/8| is bounded) and the denominator comes from a
    ones-column appended to v (lhsT free size 65).
  - causal narrowing: diagonal-region k-blocks only compute/exp/AV the
    live query range [128*i, 512); a single [128,128] triangle mask zeroes
    the partial block via one bf16 DVE multiply.
  - v / next-chunk q,k / prev-chunk output-projection matmuls are emitted
    as *fillers* between attention groups so the PE queue never drains
    while exp latency is being covered, and short-M matmuls hide their
    LDWEIGHTS under neighboring 512-row streams.
  - output projection is delayed one pair-phase so the SBUF shift DMA that
    places head-1 context into partitions 64:128 is long landed.
  - engine balance: exp + q/k evac on ACT, masks/recip/normalize/v/out
    evac on DVE, denominator broadcast + shift DMAs on Pool, loads/stores
    on the sync queue in priority order.
"""

import sys

if "/opt/trn_rl_repo" not in sys.path:
    sys.path.insert(0, "/opt/trn_rl_repo")

import numpy as np
import ml_dtypes

import concourse.bass as bass
from concourse import bacc
import concourse.mybir as mybir
import concourse.tile as tile
from concourse.bass_utils import run_bass_kernel_spmd

B, S, E, H, D = 2, 2048, 1024, 16, 64
N_CORES = 8
DP = 2                 # batch groups
TP = 4                 # cores per batch group
HL = H // TP           # local heads per core = 4
DL = HL * D            # local head dims = 256
P = 128
NTB = S // P           # token blocks = 16
QC = 512               # query chunk
NQC = S // QC          # query chunks = 4
NKB = QC // P          # k-blocks per q chunk = 4
NPAIR = HL // 2        # head pairs = 2
NEO = E // QC          # output feature chunks of 512 = 2
NKO = E // P           # contraction blocks over E = 8

f32 = mybir.dt.float32
bf16 = mybir.dt.bfloat16
EXP = mybir.ActivationFunctionType.Exp

_NC_CACHE = None


def _build_nc():
    nc = bacc.Bacc("TRN2", target_bir_lowering=False, debug=False)

    xT = nc.dram_tensor("xT", (E, S), bf16, kind="ExternalInput")
    wqT = nc.dram_tensor("wqT", (E, DL), bf16, kind="ExternalInput")
    wkT = nc.dram_tensor("wkT", (E, DL), bf16, kind="ExternalInput")
    wvT = nc.dram_tensor("wvT", (E, DL), bf16, kind="ExternalInput")
    woT = nc.dram_tensor("woT", (DL, E), bf16, kind="ExternalInput")
    out = nc.dram_tensor("out", (S, E), bf16, kind="ExternalOutput")

    with tile.TileContext(nc) as tc:
        with (
            nc.allow_low_precision(reason="bf16 operands / f32 accumulation"),
            tc.tile_pool(name="big", bufs=1) as big,
            tc.tile_pool(name="work", bufs=4) as work,
            tc.tile_pool(name="w2", bufs=4) as w2,
            tc.tile_pool(name="ps_s", bufs=2, space="PSUM") as ps_s,
            tc.tile_pool(name="ps_ctx", bufs=2, space="PSUM") as ps_ctx,
            tc.tile_pool(name="ps_mm", bufs=2, space="PSUM") as ps_mm,
        ):
            # ---- input loads (sync queue), priority order ----------------
            # per-ko weight tiles + per-(ko, chunk) x tiles so the first
            # projection chain starts as soon as its first pieces land.
            wq_t = [big.tile([P, DL], bf16, tag=f"wq{ko}", name=f"wq{ko}") for ko in range(NKO)]
            wk_t = [big.tile([P, DL], bf16, tag=f"wk{ko}", name=f"wk{ko}") for ko in range(NKO)]
            wv_t = [big.tile([P, DL], bf16, tag=f"wv{ko}", name=f"wv{ko}") for ko in range(NKO)]
            xq = [
                [big.tile([P, QC], bf16, tag=f"x{ko}_{c}", name=f"x{ko}_{c}") for c in range(NQC)]
                for ko in range(NKO)
            ]
            wqT_r = wqT[:].rearrange("(ko p) d -> ko p d", p=P)
            wkT_r = wkT[:].rearrange("(ko p) d -> ko p d", p=P)
            wvT_r = wvT[:].rearrange("(ko p) d -> ko p d", p=P)
            xT_r = xT[:].rearrange("(ko p) (c s) -> ko p c s", p=P, c=NQC)
            # weights on the scalar queue, x on the sync queue: two DMA rings
            # work the startup-critical pieces in parallel.
            for ko in range(NKO):
                nc.scalar.dma_start(wq_t[ko][:], wqT_r[ko, :, :])
                nc.sync.dma_start(xq[ko][0][:], xT_r[ko, :, 0, :])
            for ko in range(NKO):
                nc.scalar.dma_start(wk_t[ko][:], wkT_r[ko, :, :])
            for ko in range(NKO):
                nc.scalar.dma_start(wv_t[ko][:], wvT_r[ko, :, :])
            for ko in range(NKO):
                nc.sync.dma_start(xq[ko][1][:], xT_r[ko, :, 1, :])
            woT_sb = big.tile([P, NPAIR, E], bf16, tag="woT", name="woT")
            nc.scalar.dma_start(woT_sb[:], woT[:].rearrange("(pr p) e -> p pr e", p=P))
            for c in (2, 3):
                for ko in range(NKO):
                    nc.sync.dma_start(xq[ko][c][:], xT_r[ko, :, c, :])

            # ---- constants (Pool) ---------------------------------------
            # triangle mask: mask[k, q] = 1 if k <= q else 0
            mask = big.tile([P, P], bf16, tag="mask", name="mask")
            nc.gpsimd.memset(mask[:], 1.0)
            nc.gpsimd.affine_select(
                out=mask[:],
                in_=mask[:],
                compare_op=mybir.AluOpType.is_ge,
                fill=0.0,
                base=0,
                pattern=[[1, P]],
                channel_multiplier=-1,
            )

            # ---- persistent activation tiles ----------------------------
            qT_c = [[None] * NQC for _ in range(NPAIR)]
            kT_c = [[None] * NQC for _ in range(NPAIR)]
            for pr in range(NPAIR):
                for ch in range(NQC):
                    qT_c[pr][ch] = big.tile([P, QC], bf16, tag=f"qT{pr}{ch}", name=f"qT{pr}{ch}")
                    kT_c[pr][ch] = big.tile([P, QC], bf16, tag=f"kT{pr}{ch}", name=f"kT{pr}{ch}")
            v_tb = []
            for tb in range(NTB):
                vt = big.tile([P, HL, D + 1], bf16, tag=f"v{tb}", name=f"v{tb}")
                nc.gpsimd.memset(vt[:, :, D], 1.0)
                v_tb.append(vt)
            ctx_J = [
                big.tile([P, NPAIR, QC], bf16, tag=f"ctxT{J}", name=f"ctxT{J}") for J in range(NQC)
            ]

            # ---- filler generators (one PE matmul per yield) ------------
            def gen_qk(ch):
                for pr in range(NPAIR):
                    for wt, dst in ((wq_t, qT_c), (wk_t, kT_c)):
                        pp = ps_mm.tile([P, QC], f32, tag="mm", name=f"pqk{pr}{ch}")
                        for ko in range(NKO):
                            nc.tensor.matmul(
                                pp[:],
                                wt[ko][:, pr * P : (pr + 1) * P],
                                xq[ko][ch][:],
                                start=(ko == 0),
                                stop=(ko == NKO - 1),
                            )
                            yield
                        nc.vector.tensor_copy(dst[pr][ch][:], pp[:])

            def gen_v(tb0, tb1):
                for tb in range(tb0, tb1):
                    pv_full = ps_mm.tile([P, QC], f32, tag="mm", name=f"pv{tb}")
                    pv = pv_full[:, 0:DL]
                    tsl = slice((tb % NKB) * P, (tb % NKB + 1) * P)
                    for ko in range(NKO):
                        nc.tensor.matmul(
                            pv[:],
                            xq[ko][tb // NKB][:, tsl],
                            wv_t[ko][:],
                            start=(ko == 0),
                            stop=(ko == NKO - 1),
                        )
                        yield
                    nc.vector.tensor_copy(
                        v_tb[tb][:, :, 0:D],
                        pv[:].rearrange("p (h d) -> p h d", h=HL),
                    )

            def gen_out(J):
                for tb in range(NKB * J, NKB * (J + 1)):
                    tsl = slice((tb % NKB) * P, (tb % NKB + 1) * P)
                    for ec in range(NEO):
                        o_ps = ps_mm.tile([P, QC], f32, tag="mm", name=f"o{tb}{ec}")
                        for pr in range(NPAIR):
                            nc.tensor.matmul(
                                o_ps[:],
                                ctx_J[J][:, pr, tsl],
                                woT_sb[:, pr, ec * QC : (ec + 1) * QC],
                                start=(pr == 0),
                                stop=(pr == NPAIR - 1),
                            )
                            yield
                        o_sb = w2.tile([P, QC], bf16, tag="osb", name=f"ob{tb}{ec}")
                        nc.vector.tensor_copy(o_sb[:], o_ps[:])
                        nc.sync.dma_start(
                            out[tb * P : (tb + 1) * P, ec * QC : (ec + 1) * QC],
                            o_sb[:],
                        )

            # filler queue: list of (kind, chunk, generator)
            fillers = []

            def drain(n):
                k = 0
                while fillers and k < n:
                    try:
                        next(fillers[0][2])
                        k += 1
                    except StopIteration:
                        fillers.pop(0)

            def flush(kind, upto):
                i = 0
                while i < len(fillers):
                    if fillers[i][0] == kind and fillers[i][1] <= upto:
                        for _ in fillers[i][2]:
                            pass
                        fillers.pop(i)
                    else:
                        i += 1

            def flush_all():
                while fillers:
                    for _ in fillers[0][2]:
                        pass
                    fillers.pop(0)

            # ---- attention ---------------------------------------------
            def scores_group(pr, J, I):
                """Scores + exp (+ triangle mask) for k-block I of (pr, J).

                Returns (pT, qoff): exp'd probabilities, transposed
                [keys, head, queries], valid for columns [qoff:QC)."""
                kch, ib = divmod(I, NKB)
                ik = slice(ib * P, (ib + 1) * P)
                di = I - NKB * J
                qoff = di * P if di >= 0 else 0
                s = ps_s.tile([P, 2, QC], f32, tag="s", name=f"s{pr}{J}{I}")
                nc.tensor.matmul(
                    s[:, 0, qoff:],
                    kT_c[pr][kch][0:64, ik],
                    qT_c[pr][J][0:64, qoff:],
                    start=True,
                    stop=True,
                )
                nc.tensor.matmul(
                    s[:, 1, qoff:],
                    kT_c[pr][kch][64:128, ik],
                    qT_c[pr][J][64:128, qoff:],
                    start=True,
                    stop=True,
                )
                pT = work.tile([P, 2, QC], bf16, tag="pT", name=f"pT{pr}{J}{I}")
                nc.scalar.activation(pT[:, :, qoff:], s[:, :, qoff:], EXP, scale=0.125)
                if di >= 0:
                    nc.vector.tensor_tensor(
                        pT[:, :, qoff : qoff + P],
                        pT[:, :, qoff : qoff + P],
                        mask[:, None, :].to_broadcast((P, 2, P)),
                        mybir.AluOpType.mult,
                    )
                return pT, qoff

            def normalize(ctx_ps, pr, r, J):
                """ctx_J[h-half] = ctx[0:64] / ctx[64] (bf16).

                r=1 goes through an SBUF shift DMA into partitions 64:128."""
                # reciprocal_approx_fast reads garbage from PSUM (measured on
                # HW) — stage the denominator row through SBUF first.
                dn = w2.tile([1, QC], f32, tag="dn", name=f"dn{pr}{r}{J}")
                nc.vector.tensor_copy(dn[:], ctx_ps[D : D + 1, :])
                rc = w2.tile([1, QC], f32, tag="rc", name=f"rc{pr}{r}{J}")
                nc.vector.reciprocal_approx_fast(rc[:], dn[:])
                rb = w2.tile([64, QC], f32, tag="rb", name=f"rb{pr}{r}{J}")
                nc.gpsimd.partition_broadcast(rb[:], rc[:])
                if r == 0:
                    nc.vector.tensor_tensor(
                        ctx_J[J][0:64, pr, :],
                        ctx_ps[0:D, :],
                        rb[:],
                        mybir.AluOpType.mult,
                    )
                else:
                    tmp = w2.tile([64, QC], bf16, tag="tmp", name=f"ct{pr}{J}")
                    nc.vector.tensor_tensor(
                        tmp[:], ctx_ps[0:D, :], rb[:], mybir.AluOpType.mult
                    )
                    nc.gpsimd.dma_start(ctx_J[J][64:128, pr, :], tmp[:])

            def emit_attn_pair(pr, J):
                """Attention for head pair (2pr, 2pr+1) on query chunk J.

                AVs are skewed one k-block behind the scores so the in-order
                PE queue never waits on exp/mask; fillers emitted between
                groups keep the PE dense and deepen the skew."""
                h0, h1 = 2 * pr, 2 * pr + 1
                # diagonal (narrow-M) k-blocks FIRST: their exp latency is
                # covered by plentiful early fillers, and the pair tail is
                # then all full-width groups whose skew self-hides exp.
                order = list(range(NKB * J, NKB * (J + 1))) + list(range(NKB * J))
                ctx0 = ps_ctx.tile([D + 1, QC], f32, tag="ctx", name=f"c0_{pr}{J}")
                ctx1 = ps_ctx.tile([D + 1, QC], f32, tag="ctx", name=f"c1_{pr}{J}")

                def emit_av(pos, I, pT, qoff):
                    first, last = pos == 0, pos == len(order) - 1
                    nc.tensor.matmul(
                        ctx0[:, qoff:], v_tb[I][:, h0, :], pT[:, 0, qoff:],
                        start=first, stop=last,
                    )
                    nc.tensor.matmul(
                        ctx1[:, qoff:], v_tb[I][:, h1, :], pT[:, 1, qoff:],
                        start=first, stop=last,
                    )

                prev = pending.pop() if pending else scores_group(pr, J, order[0])
                for pos in range(1, len(order)):
                    cur = scores_group(pr, J, order[pos])
                    drain(4)
                    emit_av(pos - 1, order[pos - 1], *prev)
                    drain(3)
                    prev = cur
                # prefetch the NEXT pair's first scores group before the last
                # AV + normalize so the PE queue never drains at pair starts
                nxt = chain.pop(0) if chain else None
                if nxt is not None:
                    if nxt[1] != J:
                        flush("qk", nxt[1])
                    pending.append(scores_group(nxt[0], nxt[1], NKB * nxt[1]))
                emit_av(len(order) - 1, order[-1], *prev)
                normalize(ctx1, pr, 1, J)
                normalize(ctx0, pr, 0, J)

            # ---- schedule ----------------------------------------------
            chain = [(0, 0), (1, 0), (0, 1), (1, 1), (0, 2), (1, 2), (0, 3), (1, 3)]
            pending = []
            chain.pop(0)

            for _ in gen_qk(0):
                pass
            for _ in gen_v(0, NKB):
                pass
            fillers.append(("qk", 1, gen_qk(1)))
            fillers.append(("v", 1, gen_v(NKB, 2 * NKB)))
            for J in range(NQC):
                flush("v", J)
                emit_attn_pair(0, J)
                emit_attn_pair(1, J)
                if J + 2 <= NQC - 1:
                    fillers.append(("qk", J + 2, gen_qk(J + 2)))
                    fillers.append(
                        ("v", J + 2, gen_v(NKB * (J + 2), NKB * (J + 3)))
                    )
                if J < NQC - 1:
                    fillers.append(("out", J, gen_out(J)))
            flush_all()
            for _ in gen_out(NQC - 1):
                pass

    nc.compile()
    return nc


def get_nc():
    global _NC_CACHE
    if _NC_CACHE is None:
        _NC_CACHE = _build_nc()
    return _NC_CACHE


def make_in_maps(x, Wq, Wk, Wv, Wo):
    bf = ml_dtypes.bfloat16
    x = np.asarray(x, dtype=np.float32)
    Wq = np.asarray(Wq, dtype=np.float32)
    Wk = np.asarray(Wk, dtype=np.float32)
    Wv = np.asarray(Wv, dtype=np.float32)
    Wo = np.asarray(Wo, dtype=np.float32)
    in_maps = []
    for c in range(N_CORES):
        b, g = divmod(c, TP)
        sl = slice(DL * g, DL * (g + 1))
        in_maps.append(
            {
                "xT": np.ascontiguousarray(x[b].T).astype(bf),
                "wqT": np.ascontiguousarray(Wq[sl].T).astype(bf),
                "wkT": np.ascontiguousarray(Wk[sl].T).astype(bf),
                "wvT": np.ascontiguousarray(Wv[sl].T).astype(bf),
                "woT": np.ascontiguousarray(Wo[:, sl].T).astype(bf),
            }
        )
    return in_maps


def _combine(results, bo):
    bo = np.asarray(bo, dtype=np.float32)
    y = np.zeros((B, S, E), dtype=np.float32)
    for c in range(N_CORES):
        y[c // TP] += results[c]["out"].astype(np.float32)
    y += bo
    return y


def kernel(x, Wq, Wk, Wv, Wo, bo):
    nc = get_nc()
    in_maps = make_in_maps(x, Wq, Wk, Wv, Wo)
    res = run_bass_kernel_spmd(nc, in_maps, list(range(N_CORES)))
    return _combine(res.results, bo)


def kernel_traced(x, Wq, Wk, Wv, Wo, bo, trace_cores=None):
    """Like kernel() but with NTFF tracing; returns (output, BassKernelResults)."""
    nc = get_nc()
    in_maps = make_in_maps(x, Wq, Wk, Wv, Wo)
    res = run_bass_kernel_spmd(
        nc, in_maps, list(range(N_CORES)), trace=True, trace_cores=trace_cores
    )
    return _combine(res.results, bo), res


# revision 11
# speedup vs baseline: 1.0187x; 1.0187x over previous
"""Trainium2 Bass kernel for nn_MultiHeadAttention_55894704390646.

Multi-head causal attention, B=2, S=2048, E=1024, H=16 heads, D=64.
Sharding: data-parallel over batch (2 groups) x tensor-parallel over heads
(4 heads per core). Each core computes a partial output-projection result
(row-split Wo); the host sums the 4 partials per batch and adds the bias.

v2 design (all matmul operands bf16, f32 PSUM accumulation):
  - host supplies x^T [E, S] and pre-transposed weight slices in bf16, so
    every matmul contraction dim lands on SBUF partitions with no on-device
    transposes, and input DMA bytes are halved.
  - scores computed transposed ([keys, queries]); softmax uses exp(s/8)
    (no max subtraction: |s/8| is bounded) and the denominator comes from a
    ones-column appended to v (lhsT free size 65).
  - causal narrowing: diagonal-region k-blocks only compute/exp/AV the
    live query range [128*i, 512); a single [128,128] triangle mask zeroes
    the partial block via one bf16 DVE multiply.
  - v / next-chunk q,k / prev-chunk output-projection matmuls are emitted
    as *fillers* between attention groups so the PE queue never drains
    while exp latency is being covered, and short-M matmuls hide their
    LDWEIGHTS under neighboring 512-row streams.
  - output projection is delayed one pair-phase so the SBUF shift DMA that
    places head-1 context into partitions 64:128 is long landed.
  - engine balance: exp + q/k evac on ACT, masks/recip/normalize/v/out
    evac on DVE, denominator broadcast + shift DMAs on Pool, loads/stores
    on the sync queue in priority order.
"""

import sys

if "/opt/trn_rl_repo" not in sys.path:
    sys.path.insert(0, "/opt/trn_rl_repo")

import numpy as np
import ml_dtypes

import concourse.bass as bass
from concourse import bacc
import concourse.mybir as mybir
import concourse.tile as tile
from concourse.bass_utils import run_bass_kernel_spmd

B, S, E, H, D = 2, 2048, 1024, 16, 64
N_CORES = 8
DP = 2                 # batch groups
TP = 4                 # cores per batch group
HL = H // TP           # local heads per core = 4
DL = HL * D            # local head dims = 256
P = 128
NTB = S // P           # token blocks = 16
QC = 512               # query chunk
NQC = S // QC          # query chunks = 4
NKB = QC // P          # k-blocks per q chunk = 4
NPAIR = HL // 2        # head pairs = 2
NEO = E // QC          # output feature chunks of 512 = 2
NKO = E // P           # contraction blocks over E = 8

f32 = mybir.dt.float32
bf16 = mybir.dt.bfloat16
EXP = mybir.ActivationFunctionType.Exp

_NC_CACHE = None


def _build_nc():
    nc = bacc.Bacc("TRN2", target_bir_lowering=False, debug=False)

    xT = nc.dram_tensor("xT", (E, S), bf16, kind="ExternalInput")
    wqT = nc.dram_tensor("wqT", (E, DL), bf16, kind="ExternalInput")
    wkT = nc.dram_tensor("wkT", (E, DL), bf16, kind="ExternalInput")
    wvT = nc.dram_tensor("wvT", (E, DL), bf16, kind="ExternalInput")
    woT = nc.dram_tensor("woT", (DL, E), bf16, kind="ExternalInput")
    out = nc.dram_tensor("out", (S, E), bf16, kind="ExternalOutput")

    with tile.TileContext(nc) as tc:
        with (
            nc.allow_low_precision(reason="bf16 operands / f32 accumulation"),
            tc.tile_pool(name="big", bufs=1) as big,
            tc.tile_pool(name="work", bufs=4) as work,
            tc.tile_pool(name="w2", bufs=4) as w2,
            tc.tile_pool(name="ps_s", bufs=2, space="PSUM") as ps_s,
            tc.tile_pool(name="ps_ctx", bufs=2, space="PSUM") as ps_ctx,
            tc.tile_pool(name="ps_mm", bufs=2, space="PSUM") as ps_mm,
        ):
            # ---- input loads (sync queue), priority order ----------------
            # per-ko weight tiles + per-(ko, chunk) x tiles so the first
            # projection chain starts as soon as its first pieces land.
            wq_t = [big.tile([P, DL], bf16, tag=f"wq{ko}", name=f"wq{ko}") for ko in range(NKO)]
            wk_t = [big.tile([P, DL], bf16, tag=f"wk{ko}", name=f"wk{ko}") for ko in range(NKO)]
            wv_t = [big.tile([P, DL], bf16, tag=f"wv{ko}", name=f"wv{ko}") for ko in range(NKO)]
            xq = [
                [big.tile([P, QC], bf16, tag=f"x{ko}_{c}", name=f"x{ko}_{c}") for c in range(NQC)]
                for ko in range(NKO)
            ]
            wqT_r = wqT[:].rearrange("(ko p) d -> ko p d", p=P)
            wkT_r = wkT[:].rearrange("(ko p) d -> ko p d", p=P)
            wvT_r = wvT[:].rearrange("(ko p) d -> ko p d", p=P)
            xT_r = xT[:].rearrange("(ko p) (c s) -> ko p c s", p=P, c=NQC)
            for ko in range(NKO):
                nc.sync.dma_start(wq_t[ko][:], wqT_r[ko, :, :])
                nc.sync.dma_start(xq[ko][0][:], xT_r[ko, :, 0, :])
            for ko in range(NKO):
                nc.sync.dma_start(wk_t[ko][:], wkT_r[ko, :, :])
            for ko in range(NKO):
                nc.sync.dma_start(wv_t[ko][:], wvT_r[ko, :, :])
            for ko in range(NKO):
                nc.sync.dma_start(xq[ko][1][:], xT_r[ko, :, 1, :])
            woT_sb = big.tile([P, NPAIR, E], bf16, tag="woT", name="woT")
            nc.sync.dma_start(woT_sb[:], woT[:].rearrange("(pr p) e -> p pr e", p=P))
            for c in (2, 3):
                for ko in range(NKO):
                    nc.sync.dma_start(xq[ko][c][:], xT_r[ko, :, c, :])

            # ---- constants (Pool) ---------------------------------------
            # triangle mask: mask[k, q] = 1 if k <= q else 0
            mask = big.tile([P, P], bf16, tag="mask", name="mask")
            nc.gpsimd.memset(mask[:], 1.0)
            nc.gpsimd.affine_select(
                out=mask[:],
                in_=mask[:],
                compare_op=mybir.AluOpType.is_ge,
                fill=0.0,
                base=0,
                pattern=[[1, P]],
                channel_multiplier=-1,
            )

            # ---- persistent activation tiles ----------------------------
            qT_c = [[None] * NQC for _ in range(NPAIR)]
            kT_c = [[None] * NQC for _ in range(NPAIR)]
            for pr in range(NPAIR):
                for ch in range(NQC):
                    qT_c[pr][ch] = big.tile([P, QC], bf16, tag=f"qT{pr}{ch}", name=f"qT{pr}{ch}")
                    kT_c[pr][ch] = big.tile([P, QC], bf16, tag=f"kT{pr}{ch}", name=f"kT{pr}{ch}")
            v_tb = []
            for tb in range(NTB):
                vt = big.tile([P, HL, D + 1], bf16, tag=f"v{tb}", name=f"v{tb}")
                nc.gpsimd.memset(vt[:, :, D], 1.0)
                v_tb.append(vt)
            ctx_J = [
                big.tile([P, NPAIR, QC], bf16, tag=f"ctxT{J}", name=f"ctxT{J}") for J in range(NQC)
            ]

            # ---- filler generators (one PE matmul per yield) ------------
            def gen_qk(ch):
                for pr in range(NPAIR):
                    for wt, dst in ((wq_t, qT_c), (wk_t, kT_c)):
                        pp = ps_mm.tile([P, QC], f32, tag="mm", name=f"pqk{pr}{ch}")
                        for ko in range(NKO):
                            nc.tensor.matmul(
                                pp[:],
                                wt[ko][:, pr * P : (pr + 1) * P],
                                xq[ko][ch][:],
                                start=(ko == 0),
                                stop=(ko == NKO - 1),
                            )
                            yield
                        nc.scalar.copy(dst[pr][ch][:], pp[:])

            def gen_v(tb0, tb1):
                for tb in range(tb0, tb1):
                    pv_full = ps_mm.tile([P, QC], f32, tag="mm", name=f"pv{tb}")
                    pv = pv_full[:, 0:DL]
                    tsl = slice((tb % NKB) * P, (tb % NKB + 1) * P)
                    for ko in range(NKO):
                        nc.tensor.matmul(
                            pv[:],
                            xq[ko][tb // NKB][:, tsl],
                            wv_t[ko][:],
                            start=(ko == 0),
                            stop=(ko == NKO - 1),
                        )
                        yield
                    nc.vector.tensor_copy(
                        v_tb[tb][:, :, 0:D],
                        pv[:].rearrange("p (h d) -> p h d", h=HL),
                    )

            def gen_out(J):
                for tb in range(NKB * J, NKB * (J + 1)):
                    tsl = slice((tb % NKB) * P, (tb % NKB + 1) * P)
                    for ec in range(NEO):
                        o_ps = ps_mm.tile([P, QC], f32, tag="mm", name=f"o{tb}{ec}")
                        for pr in range(NPAIR):
                            nc.tensor.matmul(
                                o_ps[:],
                                ctx_J[J][:, pr, tsl],
                                woT_sb[:, pr, ec * QC : (ec + 1) * QC],
                                start=(pr == 0),
                                stop=(pr == NPAIR - 1),
                            )
                            yield
                        o_sb = w2.tile([P, QC], bf16, tag="osb", name=f"ob{tb}{ec}")
                        nc.vector.tensor_copy(o_sb[:], o_ps[:])
                        nc.sync.dma_start(
                            out[tb * P : (tb + 1) * P, ec * QC : (ec + 1) * QC],
                            o_sb[:],
                        )

            # filler queue: list of (kind, chunk, generator)
            fillers = []

            def drain(n):
                k = 0
                while fillers and k < n:
                    try:
                        next(fillers[0][2])
                        k += 1
                    except StopIteration:
                        fillers.pop(0)

            def flush(kind, upto):
                i = 0
                while i < len(fillers):
                    if fillers[i][0] == kind and fillers[i][1] <= upto:
                        for _ in fillers[i][2]:
                            pass
                        fillers.pop(i)
                    else:
                        i += 1

            def flush_all():
                while fillers:
                    for _ in fillers[0][2]:
                        pass
                    fillers.pop(0)

            # ---- attention ---------------------------------------------
            def scores_group(pr, J, I):
                """Scores + exp (+ triangle mask) for k-block I of (pr, J).

                Returns (pT, qoff): exp'd probabilities, transposed
                [keys, head, queries], valid for columns [qoff:QC)."""
                kch, ib = divmod(I, NKB)
                ik = slice(ib * P, (ib + 1) * P)
                di = I - NKB * J
                qoff = di * P if di >= 0 else 0
                s = ps_s.tile([P, 2, QC], f32, tag="s", name=f"s{pr}{J}{I}")
                nc.tensor.matmul(
                    s[:, 0, qoff:],
                    kT_c[pr][kch][0:64, ik],
                    qT_c[pr][J][0:64, qoff:],
                    start=True,
                    stop=True,
                )
                nc.tensor.matmul(
                    s[:, 1, qoff:],
                    kT_c[pr][kch][64:128, ik],
                    qT_c[pr][J][64:128, qoff:],
                    start=True,
                    stop=True,
                )
                pT = work.tile([P, 2, QC], bf16, tag="pT", name=f"pT{pr}{J}{I}")
                nc.scalar.activation(pT[:, :, qoff:], s[:, :, qoff:], EXP, scale=0.125)
                if di >= 0:
                    nc.vector.tensor_tensor(
                        pT[:, :, qoff : qoff + P],
                        pT[:, :, qoff : qoff + P],
                        mask[:, None, :].to_broadcast((P, 2, P)),
                        mybir.AluOpType.mult,
                    )
                return pT, qoff

            def normalize(ctx_ps, pr, r, J):
                """ctx_J[h-half] = ctx[0:64] / ctx[64] (bf16).

                r=1 goes through an SBUF shift DMA into partitions 64:128."""
                # reciprocal_approx_fast reads garbage from PSUM (measured on
                # HW) — stage the denominator row through SBUF first.
                dn = w2.tile([1, QC], f32, tag="dn", name=f"dn{pr}{r}{J}")
                nc.vector.tensor_copy(dn[:], ctx_ps[D : D + 1, :])
                rc = w2.tile([1, QC], f32, tag="rc", name=f"rc{pr}{r}{J}")
                nc.vector.reciprocal_approx_fast(rc[:], dn[:])
                rb = w2.tile([64, QC], f32, tag="rb", name=f"rb{pr}{r}{J}")
                nc.gpsimd.partition_broadcast(rb[:], rc[:])
                if r == 0:
                    nc.vector.tensor_tensor(
                        ctx_J[J][0:64, pr, :],
                        ctx_ps[0:D, :],
                        rb[:],
                        mybir.AluOpType.mult,
                    )
                else:
                    tmp = w2.tile([64, QC], bf16, tag="tmp", name=f"ct{pr}{J}")
                    nc.vector.tensor_tensor(
                        tmp[:], ctx_ps[0:D, :], rb[:], mybir.AluOpType.mult
                    )
                    nc.gpsimd.dma_start(ctx_J[J][64:128, pr, :], tmp[:])

            def emit_attn_pair(pr, J):
                """Attention for head pair (2pr, 2pr+1) on query chunk J.

                AVs are skewed one k-block behind the scores so the in-order
                PE queue never waits on exp/mask; fillers emitted between
                groups keep the PE dense and deepen the skew."""
                h0, h1 = 2 * pr, 2 * pr + 1
                # diagonal (narrow-M) k-blocks FIRST: their exp latency is
                # covered by plentiful early fillers, and the pair tail is
                # then all full-width groups whose skew self-hides exp.
                order = list(range(NKB * J, NKB * (J + 1))) + list(range(NKB * J))
                ctx0 = ps_ctx.tile([D + 1, QC], f32, tag="ctx", name=f"c0_{pr}{J}")
                ctx1 = ps_ctx.tile([D + 1, QC], f32, tag="ctx", name=f"c1_{pr}{J}")

                def emit_av(pos, I, pT, qoff):
                    first, last = pos == 0, pos == len(order) - 1
                    nc.tensor.matmul(
                        ctx0[:, qoff:], v_tb[I][:, h0, :], pT[:, 0, qoff:],
                        start=first, stop=last,
                    )
                    nc.tensor.matmul(
                        ctx1[:, qoff:], v_tb[I][:, h1, :], pT[:, 1, qoff:],
                        start=first, stop=last,
                    )

                prev = pending.pop() if pending else scores_group(pr, J, order[0])
                for pos in range(1, len(order)):
                    cur = scores_group(pr, J, order[pos])
                    drain(4)
                    emit_av(pos - 1, order[pos - 1], *prev)
                    drain(3)
                    prev = cur
                # prefetch the NEXT pair's first scores group before the last
                # AV + normalize so the PE queue never drains at pair starts
                nxt = chain.pop(0) if chain else None
                if nxt is not None:
                    if nxt[1] != J:
                        flush("qk", nxt[1])
                    pending.append(scores_group(nxt[0], nxt[1], NKB * nxt[1]))
                emit_av(len(order) - 1, order[-1], *prev)
                normalize(ctx1, pr, 1, J)
                normalize(ctx0, pr, 0, J)

            # ---- schedule ----------------------------------------------
            chain = [(0, 0), (1, 0), (0, 1), (1, 1), (0, 2), (1, 2), (0, 3), (1, 3)]
            pending = []
            chain.pop(0)

            for _ in gen_qk(0):
                pass
            for _ in gen_v(0, NKB):
                pass
            fillers.append(("qk", 1, gen_qk(1)))
            fillers.append(("v", 1, gen_v(NKB, 2 * NKB)))
            for J in range(NQC):
                flush("v", J)
                emit_attn_pair(0, J)
                emit_attn_pair(1, J)
                if J + 2 <= NQC - 1:
                    fillers.append(("qk", J + 2, gen_qk(J + 2)))
                    fillers.append(
                        ("v", J + 2, gen_v(NKB * (J + 2), NKB * (J + 3)))
                    )
                if J < NQC - 1:
                    fillers.append(("out", J, gen_out(J)))
            flush_all()
            for _ in gen_out(NQC - 1):
                pass

    nc.compile()
    return nc


def get_nc():
    global _NC_CACHE
    if _NC_CACHE is None:
        _NC_CACHE = _build_nc()
    return _NC_CACHE


def make_in_maps(x, Wq, Wk, Wv, Wo):
    bf = ml_dtypes.bfloat16
    x = np.asarray(x, dtype=np.float32)
    Wq = np.asarray(Wq, dtype=np.float32)
    Wk = np.asarray(Wk, dtype=np.float32)
    Wv = np.asarray(Wv, dtype=np.float32)
    Wo = np.asarray(Wo, dtype=np.float32)
    in_maps = []
    for c in range(N_CORES):
        b, g = divmod(c, TP)
        sl = slice(DL * g, DL * (g + 1))
        in_maps.append(
            {
                "xT": np.ascontiguousarray(x[b].T).astype(bf),
                "wqT": np.ascontiguousarray(Wq[sl].T).astype(bf),
                "wkT": np.ascontiguousarray(Wk[sl].T).astype(bf),
                "wvT": np.ascontiguousarray(Wv[sl].T).astype(bf),
                "woT": np.ascontiguousarray(Wo[:, sl].T).astype(bf),
            }
        )
    return in_maps


def _combine(results, bo):
    bo = np.asarray(bo, dtype=np.float32)
    y = np.zeros((B, S, E), dtype=np.float32)
    for c in range(N_CORES):
        y[c // TP] += results[c]["out"].astype(np.float32)
    y += bo
    return y


def kernel(x, Wq, Wk, Wv, Wo, bo):
    nc = get_nc()
    in_maps = make_in_maps(x, Wq, Wk, Wv, Wo)
    res = run_bass_kernel_spmd(nc, in_maps, list(range(N_CORES)))
    return _combine(res.results, bo)


def kernel_traced(x, Wq, Wk, Wv, Wo, bo, trace_cores=None):
    """Like kernel() but with NTFF tracing; returns (output, BassKernelResults)."""
    nc = get_nc()
    in_maps = make_in_maps(x, Wq, Wk, Wv, Wo)
    res = run_bass_kernel_spmd(
        nc, in_maps, list(range(N_CORES)), trace=True, trace_cores=trace_cores
    )
    return _combine(res.results, bo), res


# revision 12
# speedup vs baseline: 1.0714x; 1.0518x over previous
"""Trainium2 Bass kernel for nn_MultiHeadAttention_55894704390646.

Multi-head causal attention, B=2, S=2048, E=1024, H=16 heads, D=64.
Sharding: data-parallel over batch (2 groups) x tensor-parallel over heads
(4 heads per core). Each core computes a partial output-projection result
(row-split Wo); the host sums the 4 partials per batch and adds the bias.

v2 design (all matmul operands bf16, f32 PSUM accumulation):
  - host supplies x^T [E, S] and pre-transposed weight slices in bf16, so
    every matmul contraction dim lands on SBUF partitions with no on-device
    transposes, and input DMA bytes are halved.
  - scores computed transposed ([keys, queries]); softmax uses exp(s/8)
    (no max subtraction: |s/8| is bounded) and the denominator comes from a
    ones-column appended to v (lhsT free size 65).
  - causal narrowing: diagonal-region k-blocks only compute/exp/AV the
    live query range [128*i, 512); a single [128,128] triangle mask zeroes
    the partial block via one bf16 DVE multiply.
  - v / next-chunk q,k / prev-chunk output-projection matmuls are emitted
    as *fillers* between attention groups so the PE queue never drains
    while exp latency is being covered, and short-M matmuls hide their
    LDWEIGHTS under neighboring 512-row streams.
  - output projection is delayed one pair-phase so the SBUF shift DMA that
    places head-1 context into partitions 64:128 is long landed.
  - engine balance: exp + q/k evac on ACT, masks/recip/normalize/v/out
    evac on DVE, denominator broadcast + shift DMAs on Pool, loads/stores
    on the sync queue in priority order.
"""

import sys

if "/opt/trn_rl_repo" not in sys.path:
    sys.path.insert(0, "/opt/trn_rl_repo")

import numpy as np
import ml_dtypes

import concourse.bass as bass
from concourse import bacc
import concourse.mybir as mybir
import concourse.tile as tile
from concourse.bass_utils import run_bass_kernel_spmd

B, S, E, H, D = 2, 2048, 1024, 16, 64
N_CORES = 8
DP = 2                 # batch groups
TP = 4                 # cores per batch group
HL = H // TP           # local heads per core = 4
DL = HL * D            # local head dims = 256
P = 128
NTB = S // P           # token blocks = 16
QC = 512               # query chunk
NQC = S // QC          # query chunks = 4
NKB = QC // P          # k-blocks per q chunk = 4
NPAIR = HL // 2        # head pairs = 2
NEO = E // QC          # output feature chunks of 512 = 2
NKO = E // P           # contraction blocks over E = 8

f32 = mybir.dt.float32
bf16 = mybir.dt.bfloat16
EXP = mybir.ActivationFunctionType.Exp

_NC_CACHE = None


def _build_nc():
    nc = bacc.Bacc("TRN2", target_bir_lowering=False, debug=False)

    xT = nc.dram_tensor("xT", (E, S), bf16, kind="ExternalInput")
    wqT = nc.dram_tensor("wqT", (E, DL), bf16, kind="ExternalInput")
    wkT = nc.dram_tensor("wkT", (E, DL), bf16, kind="ExternalInput")
    wvT = nc.dram_tensor("wvT", (E, DL), bf16, kind="ExternalInput")
    woT = nc.dram_tensor("woT", (DL, E), bf16, kind="ExternalInput")
    out = nc.dram_tensor("out", (S, E), bf16, kind="ExternalOutput")

    with tile.TileContext(nc) as tc:
        with (
            nc.allow_low_precision(reason="bf16 operands / f32 accumulation"),
            tc.tile_pool(name="big", bufs=1) as big,
            tc.tile_pool(name="work", bufs=4) as work,
            tc.tile_pool(name="w2", bufs=4) as w2,
            tc.tile_pool(name="ps_s", bufs=2, space="PSUM") as ps_s,
            tc.tile_pool(name="ps_ctx", bufs=2, space="PSUM") as ps_ctx,
            tc.tile_pool(name="ps_mm", bufs=2, space="PSUM") as ps_mm,
        ):
            # ---- input loads (sync queue), priority order ----------------
            # per-ko weight tiles + per-(ko, chunk) x tiles so the first
            # projection chain starts as soon as its first pieces land.
            wq_t = [big.tile([P, DL], bf16, tag=f"wq{ko}", name=f"wq{ko}") for ko in range(NKO)]
            wk_t = [big.tile([P, DL], bf16, tag=f"wk{ko}", name=f"wk{ko}") for ko in range(NKO)]
            wv_t = [big.tile([P, DL], bf16, tag=f"wv{ko}", name=f"wv{ko}") for ko in range(NKO)]
            xq = [
                [big.tile([P, QC], bf16, tag=f"x{ko}_{c}", name=f"x{ko}_{c}") for c in range(NQC)]
                for ko in range(NKO)
            ]
            wqT_r = wqT[:].rearrange("(ko p) d -> ko p d", p=P)
            wkT_r = wkT[:].rearrange("(ko p) d -> ko p d", p=P)
            wvT_r = wvT[:].rearrange("(ko p) d -> ko p d", p=P)
            xT_r = xT[:].rearrange("(ko p) (c s) -> ko p c s", p=P, c=NQC)
            for ko in range(NKO):
                nc.sync.dma_start(wq_t[ko][:], wqT_r[ko, :, :])
                nc.sync.dma_start(xq[ko][0][:], xT_r[ko, :, 0, :])
            for ko in range(NKO):
                nc.sync.dma_start(wk_t[ko][:], wkT_r[ko, :, :])
            for ko in range(NKO):
                nc.sync.dma_start(wv_t[ko][:], wvT_r[ko, :, :])
            for ko in range(NKO):
                nc.sync.dma_start(xq[ko][1][:], xT_r[ko, :, 1, :])
            woT_sb = big.tile([P, NPAIR, E], bf16, tag="woT", name="woT")
            nc.sync.dma_start(woT_sb[:], woT[:].rearrange("(pr p) e -> p pr e", p=P))
            for c in (2, 3):
                for ko in range(NKO):
                    nc.sync.dma_start(xq[ko][c][:], xT_r[ko, :, c, :])

            # ---- constants (Pool) ---------------------------------------
            # triangle mask: mask[k, q] = 1 if k <= q else 0
            mask = big.tile([P, P], bf16, tag="mask", name="mask")
            nc.gpsimd.memset(mask[:], 1.0)
            nc.gpsimd.affine_select(
                out=mask[:],
                in_=mask[:],
                compare_op=mybir.AluOpType.is_ge,
                fill=0.0,
                base=0,
                pattern=[[1, P]],
                channel_multiplier=-1,
            )

            # ---- persistent activation tiles ----------------------------
            qT_c = [[None] * NQC for _ in range(NPAIR)]
            kT_c = [[None] * NQC for _ in range(NPAIR)]
            for pr in range(NPAIR):
                for ch in range(NQC):
                    qT_c[pr][ch] = big.tile([P, QC], bf16, tag=f"qT{pr}{ch}", name=f"qT{pr}{ch}")
                    kT_c[pr][ch] = big.tile([P, QC], bf16, tag=f"kT{pr}{ch}", name=f"kT{pr}{ch}")
            v_tb = []
            for tb in range(NTB):
                vt = big.tile([P, HL, D + 1], bf16, tag=f"v{tb}", name=f"v{tb}")
                nc.gpsimd.memset(vt[:, :, D], 1.0)
                v_tb.append(vt)
            ctx_J = [
                big.tile([P, NPAIR, QC], bf16, tag=f"ctxT{J}", name=f"ctxT{J}") for J in range(NQC)
            ]

            # ---- filler generators (one PE matmul per yield) ------------
            def gen_qk(ch):
                for pr in range(NPAIR):
                    for wt, dst in ((wq_t, qT_c), (wk_t, kT_c)):
                        pp = ps_mm.tile([P, QC], f32, tag="mm", name=f"pqk{pr}{ch}")
                        for ko in range(NKO):
                            nc.tensor.matmul(
                                pp[:],
                                wt[ko][:, pr * P : (pr + 1) * P],
                                xq[ko][ch][:],
                                start=(ko == 0),
                                stop=(ko == NKO - 1),
                            )
                            yield
                        nc.scalar.copy(dst[pr][ch][:], pp[:])

            def gen_v(tb0, tb1):
                for tb in range(tb0, tb1):
                    pv_full = ps_mm.tile([P, QC], f32, tag="mm", name=f"pv{tb}")
                    pv = pv_full[:, 0:DL]
                    tsl = slice((tb % NKB) * P, (tb % NKB + 1) * P)
                    for ko in range(NKO):
                        nc.tensor.matmul(
                            pv[:],
                            xq[ko][tb // NKB][:, tsl],
                            wv_t[ko][:],
                            start=(ko == 0),
                            stop=(ko == NKO - 1),
                        )
                        yield
                    nc.vector.tensor_copy(
                        v_tb[tb][:, :, 0:D],
                        pv[:].rearrange("p (h d) -> p h d", h=HL),
                    )

            def gen_out(J):
                for tb in range(NKB * J, NKB * (J + 1)):
                    tsl = slice((tb % NKB) * P, (tb % NKB + 1) * P)
                    for ec in range(NEO):
                        o_ps = ps_mm.tile([P, QC], f32, tag="mm", name=f"o{tb}{ec}")
                        for pr in range(NPAIR):
                            nc.tensor.matmul(
                                o_ps[:],
                                ctx_J[J][:, pr, tsl],
                                woT_sb[:, pr, ec * QC : (ec + 1) * QC],
                                start=(pr == 0),
                                stop=(pr == NPAIR - 1),
                            )
                            yield
                        o_sb = w2.tile([P, QC], bf16, tag="osb", name=f"ob{tb}{ec}")
                        nc.vector.tensor_copy(o_sb[:], o_ps[:])
                        nc.sync.dma_start(
                            out[tb * P : (tb + 1) * P, ec * QC : (ec + 1) * QC],
                            o_sb[:],
                        )

            # filler queue: list of (kind, chunk, generator)
            fillers = []

            def drain(n):
                k = 0
                while fillers and k < n:
                    try:
                        next(fillers[0][2])
                        k += 1
                    except StopIteration:
                        fillers.pop(0)

            def flush(kind, upto):
                i = 0
                while i < len(fillers):
                    if fillers[i][0] == kind and fillers[i][1] <= upto:
                        for _ in fillers[i][2]:
                            pass
                        fillers.pop(i)
                    else:
                        i += 1

            def flush_all():
                while fillers:
                    for _ in fillers[0][2]:
                        pass
                    fillers.pop(0)

            # ---- attention ---------------------------------------------
            def scores_group(pr, J, I):
                """Scores + exp (+ triangle mask) for k-block I of (pr, J).

                Returns (pT, qoff): exp'd probabilities, transposed
                [keys, head, queries], valid for columns [qoff:QC)."""
                kch, ib = divmod(I, NKB)
                ik = slice(ib * P, (ib + 1) * P)
                di = I - NKB * J
                qoff = di * P if di >= 0 else 0
                s = ps_s.tile([P, 2, QC], f32, tag="s", name=f"s{pr}{J}{I}")
                nc.tensor.matmul(
                    s[:, 0, qoff:],
                    kT_c[pr][kch][0:64, ik],
                    qT_c[pr][J][0:64, qoff:],
                    start=True,
                    stop=True,
                )
                nc.tensor.matmul(
                    s[:, 1, qoff:],
                    kT_c[pr][kch][64:128, ik],
                    qT_c[pr][J][64:128, qoff:],
                    start=True,
                    stop=True,
                )
                pT = work.tile([P, 2, QC], bf16, tag="pT", name=f"pT{pr}{J}{I}")
                nc.scalar.activation(pT[:, :, qoff:], s[:, :, qoff:], EXP, scale=0.125)
                if di >= 0:
                    nc.vector.tensor_tensor(
                        pT[:, :, qoff : qoff + P],
                        pT[:, :, qoff : qoff + P],
                        mask[:, None, :].to_broadcast((P, 2, P)),
                        mybir.AluOpType.mult,
                    )
                return pT, qoff

            def normalize(ctx_ps, pr, r, J):
                """ctx_J[h-half] = ctx[0:64] / ctx[64] (bf16).

                r=1 goes through an SBUF shift DMA into partitions 64:128."""
                # reciprocal_approx_fast reads garbage from PSUM (measured on
                # HW) — stage the denominator row through SBUF first.
                dn = w2.tile([1, QC], f32, tag="dn", name=f"dn{pr}{r}{J}")
                nc.vector.tensor_copy(dn[:], ctx_ps[D : D + 1, :])
                rc = w2.tile([1, QC], f32, tag="rc", name=f"rc{pr}{r}{J}")
                nc.vector.reciprocal_approx_fast(rc[:], dn[:])
                rb = w2.tile([64, QC], f32, tag="rb", name=f"rb{pr}{r}{J}")
                nc.gpsimd.partition_broadcast(rb[:], rc[:])
                if r == 0:
                    nc.vector.tensor_tensor(
                        ctx_J[J][0:64, pr, :],
                        ctx_ps[0:D, :],
                        rb[:],
                        mybir.AluOpType.mult,
                    )
                else:
                    tmp = w2.tile([64, QC], bf16, tag="tmp", name=f"ct{pr}{J}")
                    nc.vector.tensor_tensor(
                        tmp[:], ctx_ps[0:D, :], rb[:], mybir.AluOpType.mult
                    )
                    nc.gpsimd.dma_start(ctx_J[J][64:128, pr, :], tmp[:])

            def emit_attn_pair(pr, J):
                """Attention for head pair (2pr, 2pr+1) on query chunk J.

                AVs are skewed one k-block behind the scores so the in-order
                PE queue never waits on exp/mask; fillers emitted between
                groups keep the PE dense and deepen the skew."""
                h0, h1 = 2 * pr, 2 * pr + 1
                # diagonal (narrow-M) k-blocks FIRST: their exp latency is
                # covered by plentiful early fillers, and the pair tail is
                # then all full-width groups whose skew self-hides exp.
                order = list(range(NKB * (J + 1)))
                ctx0 = ps_ctx.tile([D + 1, QC], f32, tag="ctx", name=f"c0_{pr}{J}")
                ctx1 = ps_ctx.tile([D + 1, QC], f32, tag="ctx", name=f"c1_{pr}{J}")

                def emit_av(pos, I, pT, qoff):
                    first, last = pos == 0, pos == len(order) - 1
                    nc.tensor.matmul(
                        ctx0[:, qoff:], v_tb[I][:, h0, :], pT[:, 0, qoff:],
                        start=first, stop=last,
                    )
                    nc.tensor.matmul(
                        ctx1[:, qoff:], v_tb[I][:, h1, :], pT[:, 1, qoff:],
                        start=first, stop=last,
                    )

                prev = pending.pop() if pending else scores_group(pr, J, order[0])
                for pos in range(1, len(order)):
                    cur = scores_group(pr, J, order[pos])
                    drain(4)
                    emit_av(pos - 1, order[pos - 1], *prev)
                    drain(3)
                    prev = cur
                # prefetch the NEXT pair's first scores group before the last
                # AV + normalize so the PE queue never drains at pair starts
                nxt = chain.pop(0) if chain else None
                if nxt is not None:
                    if nxt[1] != J:
                        flush("qk", nxt[1])
                    pending.append(scores_group(nxt[0], nxt[1], 0))
                emit_av(len(order) - 1, order[-1], *prev)
                normalize(ctx1, pr, 1, J)
                normalize(ctx0, pr, 0, J)

            # ---- schedule ----------------------------------------------
            chain = [(0, 0), (1, 0), (0, 1), (1, 1), (0, 2), (1, 2), (0, 3), (1, 3)]
            pending = []
            chain.pop(0)

            for _ in gen_qk(0):
                pass
            for _ in gen_v(0, NKB):
                pass
            fillers.append(("qk", 1, gen_qk(1)))
            fillers.append(("v", 1, gen_v(NKB, 2 * NKB)))
            for J in range(NQC):
                flush("v", J)
                emit_attn_pair(0, J)
                emit_attn_pair(1, J)
                if J + 2 <= NQC - 1:
                    fillers.append(("qk", J + 2, gen_qk(J + 2)))
                    fillers.append(
                        ("v", J + 2, gen_v(NKB * (J + 2), NKB * (J + 3)))
                    )
                if J < NQC - 1:
                    fillers.append(("out", J, gen_out(J)))
            flush_all()
            for _ in gen_out(NQC - 1):
                pass

    nc.compile()
    return nc


def get_nc():
    global _NC_CACHE
    if _NC_CACHE is None:
        _NC_CACHE = _build_nc()
    return _NC_CACHE


def make_in_maps(x, Wq, Wk, Wv, Wo):
    bf = ml_dtypes.bfloat16
    x = np.asarray(x, dtype=np.float32)
    Wq = np.asarray(Wq, dtype=np.float32)
    Wk = np.asarray(Wk, dtype=np.float32)
    Wv = np.asarray(Wv, dtype=np.float32)
    Wo = np.asarray(Wo, dtype=np.float32)
    in_maps = []
    for c in range(N_CORES):
        b, g = divmod(c, TP)
        sl = slice(DL * g, DL * (g + 1))
        in_maps.append(
            {
                "xT": np.ascontiguousarray(x[b].T).astype(bf),
                "wqT": np.ascontiguousarray(Wq[sl].T).astype(bf),
                "wkT": np.ascontiguousarray(Wk[sl].T).astype(bf),
                "wvT": np.ascontiguousarray(Wv[sl].T).astype(bf),
                "woT": np.ascontiguousarray(Wo[:, sl].T).astype(bf),
            }
        )
    return in_maps


def _combine(results, bo):
    bo = np.asarray(bo, dtype=np.float32)
    y = np.zeros((B, S, E), dtype=np.float32)
    for c in range(N_CORES):
        y[c // TP] += results[c]["out"].astype(np.float32)
    y += bo
    return y


def kernel(x, Wq, Wk, Wv, Wo, bo):
    nc = get_nc()
    in_maps = make_in_maps(x, Wq, Wk, Wv, Wo)
    res = run_bass_kernel_spmd(nc, in_maps, list(range(N_CORES)))
    return _combine(res.results, bo)


def kernel_traced(x, Wq, Wk, Wv, Wo, bo, trace_cores=None):
    """Like kernel() but with NTFF tracing; returns (output, BassKernelResults)."""
    nc = get_nc()
    in_maps = make_in_maps(x, Wq, Wk, Wv, Wo)
    res = run_bass_kernel_spmd(
        nc, in_maps, list(range(N_CORES)), trace=True, trace_cores=trace_cores
    )
    return _combine(res.results, bo), res


# revision 15
# speedup vs baseline: 1.0927x; 1.0199x over previous
"""Trainium2 Bass kernel for nn_MultiHeadAttention_55894704390646.

Multi-head causal attention, B=2, S=2048, E=1024, H=16 heads, D=64.
Sharding: data-parallel over batch (2 groups) x tensor-parallel over heads
(4 heads per core). Each core computes a partial output-projection result
(row-split Wo); the host sums the 4 partials per batch and adds the bias.

v2 design (all matmul operands bf16, f32 PSUM accumulation):
  - host supplies x^T [E, S] and pre-transposed weight slices in bf16, so
    every matmul contraction dim lands on SBUF partitions with no on-device
    transposes, and input DMA bytes are halved.
  - scores computed transposed ([keys, queries]); softmax uses exp(s/8)
    (no max subtraction: |s/8| is bounded) and the denominator comes from a
    ones-column appended to v (lhsT free size 65).
  - causal narrowing: diagonal-region k-blocks only compute/exp/AV the
    live query range [128*i, 512); a single [128,128] triangle mask zeroes
    the partial block via one bf16 DVE multiply.
  - v / next-chunk q,k / prev-chunk output-projection matmuls are emitted
    as *fillers* between attention groups so the PE queue never drains
    while exp latency is being covered, and short-M matmuls hide their
    LDWEIGHTS under neighboring 512-row streams.
  - output projection is delayed one pair-phase so the SBUF shift DMA that
    places head-1 context into partitions 64:128 is long landed.
  - engine balance: exp + q/k evac on ACT, masks/recip/normalize/v/out
    evac on DVE, denominator broadcast + shift DMAs on Pool, loads/stores
    on the sync queue in priority order.
"""

import sys

if "/opt/trn_rl_repo" not in sys.path:
    sys.path.insert(0, "/opt/trn_rl_repo")

import numpy as np
import ml_dtypes

import concourse.bass as bass
from concourse import bacc
import concourse.mybir as mybir
import concourse.tile as tile
from concourse.bass_utils import run_bass_kernel_spmd

B, S, E, H, D = 2, 2048, 1024, 16, 64
N_CORES = 8
DP = 2                 # batch groups
TP = 4                 # cores per batch group
HL = H // TP           # local heads per core = 4
DL = HL * D            # local head dims = 256
P = 128
NTB = S // P           # token blocks = 16
QC = 512               # query chunk
NQC = S // QC          # query chunks = 4
NKB = QC // P          # k-blocks per q chunk = 4
NPAIR = HL // 2        # head pairs = 2
NEO = E // QC          # output feature chunks of 512 = 2
NKO = E // P           # contraction blocks over E = 8

f32 = mybir.dt.float32
bf16 = mybir.dt.bfloat16
EXP = mybir.ActivationFunctionType.Exp

_NC_CACHE = None


def _build_nc():
    nc = bacc.Bacc("TRN2", target_bir_lowering=False, debug=False)

    xT = nc.dram_tensor("xT", (E, S), bf16, kind="ExternalInput")
    wqT = nc.dram_tensor("wqT", (E, DL), bf16, kind="ExternalInput")
    wkT = nc.dram_tensor("wkT", (E, DL), bf16, kind="ExternalInput")
    wvT = nc.dram_tensor("wvT", (E, DL), bf16, kind="ExternalInput")
    woT = nc.dram_tensor("woT", (DL, E), bf16, kind="ExternalInput")
    out = nc.dram_tensor("out", (S, E), bf16, kind="ExternalOutput")

    with tile.TileContext(nc) as tc:
        with (
            nc.allow_low_precision(reason="bf16 operands / f32 accumulation"),
            tc.tile_pool(name="big", bufs=1) as big,
            tc.tile_pool(name="work", bufs=4) as work,
            tc.tile_pool(name="w2", bufs=4) as w2,
            tc.tile_pool(name="ps_s", bufs=2, space="PSUM") as ps_s,
            tc.tile_pool(name="ps_ctx", bufs=2, space="PSUM") as ps_ctx,
            tc.tile_pool(name="ps_mm", bufs=2, space="PSUM") as ps_mm,
        ):
            # ---- input loads (sync queue), priority order ----------------
            # per-ko weight tiles + per-(ko, chunk) x tiles so the first
            # projection chain starts as soon as its first pieces land.
            wq_t = [big.tile([P, DL], bf16, tag=f"wq{ko}", name=f"wq{ko}") for ko in range(NKO)]
            wk_t = [big.tile([P, DL], bf16, tag=f"wk{ko}", name=f"wk{ko}") for ko in range(NKO)]
            wv_t = [big.tile([P, DL], bf16, tag=f"wv{ko}", name=f"wv{ko}") for ko in range(NKO)]
            xq = [
                [big.tile([P, QC], bf16, tag=f"x{ko}_{c}", name=f"x{ko}_{c}") for c in range(NQC)]
                for ko in range(NKO)
            ]
            wqT_r = wqT[:].rearrange("(ko p) d -> ko p d", p=P)
            wkT_r = wkT[:].rearrange("(ko p) d -> ko p d", p=P)
            wvT_r = wvT[:].rearrange("(ko p) d -> ko p d", p=P)
            xT_r = xT[:].rearrange("(ko p) (c s) -> ko p c s", p=P, c=NQC)
            for ko in range(NKO):
                nc.gpsimd.dma_start(wq_t[ko][:], wqT_r[ko, :, :])
                nc.sync.dma_start(xq[ko][0][:], xT_r[ko, :, 0, :])
            for ko in range(NKO):
                nc.gpsimd.dma_start(wk_t[ko][:], wkT_r[ko, :, :])
            for ko in range(NKO):
                nc.gpsimd.dma_start(wv_t[ko][:], wvT_r[ko, :, :])
            for ko in range(NKO):
                nc.sync.dma_start(xq[ko][1][:], xT_r[ko, :, 1, :])
            woT_sb = big.tile([P, NPAIR, E], bf16, tag="woT", name="woT")
            nc.gpsimd.dma_start(woT_sb[:], woT[:].rearrange("(pr p) e -> p pr e", p=P))
            for c in (2, 3):
                for ko in range(NKO):
                    nc.sync.dma_start(xq[ko][c][:], xT_r[ko, :, c, :])

            # ---- constants (Pool) ---------------------------------------
            # triangle mask: mask[k, q] = 1 if k <= q else 0
            mask = big.tile([P, P], bf16, tag="mask", name="mask")
            nc.gpsimd.memset(mask[:], 1.0)
            nc.gpsimd.affine_select(
                out=mask[:],
                in_=mask[:],
                compare_op=mybir.AluOpType.is_ge,
                fill=0.0,
                base=0,
                pattern=[[1, P]],
                channel_multiplier=-1,
            )

            # ---- persistent activation tiles ----------------------------
            qT_c = [[None] * NQC for _ in range(NPAIR)]
            kT_c = [[None] * NQC for _ in range(NPAIR)]
            for pr in range(NPAIR):
                for ch in range(NQC):
                    qT_c[pr][ch] = big.tile([P, QC], bf16, tag=f"qT{pr}{ch}", name=f"qT{pr}{ch}")
                    kT_c[pr][ch] = big.tile([P, QC], bf16, tag=f"kT{pr}{ch}", name=f"kT{pr}{ch}")
            v_tb = []
            for tb in range(NTB):
                vt = big.tile([P, HL, D + 1], bf16, tag=f"v{tb}", name=f"v{tb}")
                nc.gpsimd.memset(vt[:, :, D], 1.0)
                v_tb.append(vt)
            ctx_J = [
                big.tile([P, NPAIR, QC], bf16, tag=f"ctxT{J}", name=f"ctxT{J}") for J in range(NQC)
            ]

            # ---- filler generators (one PE matmul per yield) ------------
            def gen_qk(ch):
                for pr in range(NPAIR):
                    for wt, dst in ((wq_t, qT_c), (wk_t, kT_c)):
                        pp = ps_mm.tile([P, QC], f32, tag="mm", name=f"pqk{pr}{ch}")
                        for ko in range(NKO):
                            nc.tensor.matmul(
                                pp[:],
                                wt[ko][:, pr * P : (pr + 1) * P],
                                xq[ko][ch][:],
                                start=(ko == 0),
                                stop=(ko == NKO - 1),
                            )
                            yield
                        nc.scalar.copy(dst[pr][ch][:], pp[:])

            def gen_v(tb0, tb1):
                for tb in range(tb0, tb1):
                    pv_full = ps_mm.tile([P, QC], f32, tag="mm", name=f"pv{tb}")
                    pv = pv_full[:, 0:DL]
                    tsl = slice((tb % NKB) * P, (tb % NKB + 1) * P)
                    for ko in range(NKO):
                        nc.tensor.matmul(
                            pv[:],
                            xq[ko][tb // NKB][:, tsl],
                            wv_t[ko][:],
                            start=(ko == 0),
                            stop=(ko == NKO - 1),
                        )
                        yield
                    nc.vector.tensor_copy(
                        v_tb[tb][:, :, 0:D],
                        pv[:].rearrange("p (h d) -> p h d", h=HL),
                    )

            def gen_out(J):
                for tb in range(NKB * J, NKB * (J + 1)):
                    tsl = slice((tb % NKB) * P, (tb % NKB + 1) * P)
                    for ec in range(NEO):
                        o_ps = ps_mm.tile([P, QC], f32, tag="mm", name=f"o{tb}{ec}")
                        for pr in range(NPAIR):
                            nc.tensor.matmul(
                                o_ps[:],
                                ctx_J[J][:, pr, tsl],
                                woT_sb[:, pr, ec * QC : (ec + 1) * QC],
                                start=(pr == 0),
                                stop=(pr == NPAIR - 1),
                            )
                            yield
                        o_sb = w2.tile([P, QC], bf16, tag="osb", name=f"ob{tb}{ec}")
                        nc.vector.tensor_copy(o_sb[:], o_ps[:])
                        nc.sync.dma_start(
                            out[tb * P : (tb + 1) * P, ec * QC : (ec + 1) * QC],
                            o_sb[:],
                        )

            # filler queue: list of [kind, chunk, generator, remaining]
            fillers = []
            reserve = [0]  # keep >= this many filler matmuls for the tail

            def remaining():
                return sum(f[3] for f in fillers)

            def drain(n):
                k = 0
                while fillers and k < n and remaining() > reserve[0]:
                    try:
                        next(fillers[0][2])
                        fillers[0][3] -= 1
                        k += 1
                    except StopIteration:
                        fillers.pop(0)

            def flush(kind, upto):
                i = 0
                while i < len(fillers):
                    if fillers[i][0] == kind and fillers[i][1] <= upto:
                        for _ in fillers[i][2]:
                            pass
                        fillers.pop(i)
                    else:
                        i += 1

            def flush_all():
                while fillers:
                    for _ in fillers[0][2]:
                        pass
                    fillers.pop(0)

            # ---- attention ---------------------------------------------
            def scores_group(pr, J, I):
                """Scores + exp (+ triangle mask) for k-block I of (pr, J).

                Returns (pT, qoff): exp'd probabilities, transposed
                [keys, head, queries], valid for columns [qoff:QC)."""
                kch, ib = divmod(I, NKB)
                ik = slice(ib * P, (ib + 1) * P)
                di = I - NKB * J
                qoff = di * P if di >= 0 else 0
                s = ps_s.tile([P, 2, QC], f32, tag="s", name=f"s{pr}{J}{I}")
                nc.tensor.matmul(
                    s[:, 0, qoff:],
                    kT_c[pr][kch][0:64, ik],
                    qT_c[pr][J][0:64, qoff:],
                    start=True,
                    stop=True,
                )
                nc.tensor.matmul(
                    s[:, 1, qoff:],
                    kT_c[pr][kch][64:128, ik],
                    qT_c[pr][J][64:128, qoff:],
                    start=True,
                    stop=True,
                )
                pT = work.tile([P, 2, QC], bf16, tag="pT", name=f"pT{pr}{J}{I}")
                nc.scalar.activation(pT[:, :, qoff:], s[:, :, qoff:], EXP, scale=0.125)
                if di >= 0:
                    nc.vector.tensor_tensor(
                        pT[:, :, qoff : qoff + P],
                        pT[:, :, qoff : qoff + P],
                        mask[:, None, :].to_broadcast((P, 2, P)),
                        mybir.AluOpType.mult,
                    )
                return pT, qoff

            def normalize(ctx_ps, pr, r, J):
                """ctx_J[h-half] = ctx[0:64] / ctx[64] (bf16).

                r=1 goes through an SBUF shift DMA into partitions 64:128."""
                # reciprocal_approx_fast reads garbage from PSUM (measured on
                # HW) — stage the denominator row through SBUF first.
                dn = w2.tile([1, QC], f32, tag="dn", name=f"dn{pr}{r}{J}")
                nc.vector.tensor_copy(dn[:], ctx_ps[D : D + 1, :])
                rc = w2.tile([1, QC], f32, tag="rc", name=f"rc{pr}{r}{J}")
                nc.vector.reciprocal_approx_fast(rc[:], dn[:])
                rb = w2.tile([64, QC], f32, tag="rb", name=f"rb{pr}{r}{J}")
                nc.gpsimd.partition_broadcast(rb[:], rc[:])
                if r == 0:
                    nc.vector.tensor_tensor(
                        ctx_J[J][0:64, pr, :],
                        ctx_ps[0:D, :],
                        rb[:],
                        mybir.AluOpType.mult,
                    )
                else:
                    tmp = w2.tile([64, QC], bf16, tag="tmp", name=f"ct{pr}{J}")
                    nc.vector.tensor_tensor(
                        tmp[:], ctx_ps[0:D, :], rb[:], mybir.AluOpType.mult
                    )
                    nc.gpsimd.dma_start(ctx_J[J][64:128, pr, :], tmp[:])

            def emit_attn_pair(pr, J):
                """Attention for head pair (2pr, 2pr+1) on query chunk J.

                AVs are skewed one k-block behind the scores so the in-order
                PE queue never waits on exp/mask; fillers emitted between
                groups keep the PE dense and deepen the skew."""
                h0, h1 = 2 * pr, 2 * pr + 1
                order = list(range(NKB * (J + 1)))
                ctx0 = ps_ctx.tile([D + 1, QC], f32, tag="ctx", name=f"c0_{pr}{J}")
                ctx1 = ps_ctx.tile([D + 1, QC], f32, tag="ctx", name=f"c1_{pr}{J}")

                def emit_av(pos, I, pT, qoff, mid_drain=0):
                    first, last = pos == 0, pos == len(order) - 1
                    nc.tensor.matmul(
                        ctx0[:, qoff:], v_tb[I][:, h0, :], pT[:, 0, qoff:],
                        start=first, stop=last,
                    )
                    if mid_drain:
                        drain(mid_drain)
                    nc.tensor.matmul(
                        ctx1[:, qoff:], v_tb[I][:, h1, :], pT[:, 1, qoff:],
                        start=first, stop=last,
                    )

                prev = pending.pop() if pending else scores_group(pr, J, order[0])
                for pos in range(1, len(order)):
                    cur = scores_group(pr, J, order[pos])
                    drain(3)
                    emit_av(pos - 1, order[pos - 1], *prev, mid_drain=1)
                    drain(3)
                    prev = cur
                # prefetch the NEXT pair's first scores group before the last
                # AV + normalize so the PE queue never drains at pair starts
                nxt = chain.pop(0) if chain else None
                if nxt is not None:
                    if nxt[1] != J:
                        flush("qk", nxt[1])
                    pending.append(scores_group(nxt[0], nxt[1], 0))
                emit_av(len(order) - 1, order[-1], *prev)
                normalize(ctx1, pr, 1, J)
                normalize(ctx0, pr, 0, J)

            # ---- schedule ----------------------------------------------
            chain = [(0, 0), (1, 0), (0, 1), (1, 1), (0, 2), (1, 2), (0, 3), (1, 3)]
            pending = []
            chain.pop(0)

            for _ in gen_qk(0):
                pass
            for _ in gen_v(0, NKB):
                pass
            fillers.append(["qk", 1, gen_qk(1), 32])
            fillers.append(["v", 1, gen_v(NKB, 2 * NKB), 32])
            for J in range(NQC):
                # during the last chunk, hold back fillers so the PE has work
                # while the final normalize/shift-DMA chain completes
                reserve[0] = 8 if J == NQC - 1 else 0
                flush("v", J)
                emit_attn_pair(0, J)
                emit_attn_pair(1, J)
                if J + 2 <= NQC - 1:
                    fillers.append(["qk", J + 2, gen_qk(J + 2), 32])
                    fillers.append(
                        ["v", J + 2, gen_v(NKB * (J + 2), NKB * (J + 3)), 32]
                    )
                if J < NQC - 1:
                    fillers.append(["out", J, gen_out(J), 16])
            reserve[0] = 0
            flush_all()
            for _ in gen_out(NQC - 1):
                pass

    nc.compile()
    return nc


def get_nc():
    global _NC_CACHE
    if _NC_CACHE is None:
        _NC_CACHE = _build_nc()
    return _NC_CACHE


def make_in_maps(x, Wq, Wk, Wv, Wo):
    bf = ml_dtypes.bfloat16
    x = np.asarray(x, dtype=np.float32)
    Wq = np.asarray(Wq, dtype=np.float32)
    Wk = np.asarray(Wk, dtype=np.float32)
    Wv = np.asarray(Wv, dtype=np.float32)
    Wo = np.asarray(Wo, dtype=np.float32)
    in_maps = []
    for c in range(N_CORES):
        b, g = divmod(c, TP)
        sl = slice(DL * g, DL * (g + 1))
        in_maps.append(
            {
                "xT": np.ascontiguousarray(x[b].T).astype(bf),
                "wqT": np.ascontiguousarray(Wq[sl].T).astype(bf),
                "wkT": np.ascontiguousarray(Wk[sl].T).astype(bf),
                "wvT": np.ascontiguousarray(Wv[sl].T).astype(bf),
                "woT": np.ascontiguousarray(Wo[:, sl].T).astype(bf),
            }
        )
    return in_maps


def _combine(results, bo):
    bo = np.asarray(bo, dtype=np.float32)
    y = np.zeros((B, S, E), dtype=np.float32)
    for c in range(N_CORES):
        y[c // TP] += results[c]["out"].astype(np.float32)
    y += bo
    return y


def kernel(x, Wq, Wk, Wv, Wo, bo):
    nc = get_nc()
    in_maps = make_in_maps(x, Wq, Wk, Wv, Wo)
    res = run_bass_kernel_spmd(nc, in_maps, list(range(N_CORES)))
    return _combine(res.results, bo)


def kernel_traced(x, Wq, Wk, Wv, Wo, bo, trace_cores=None):
    """Like kernel() but with NTFF tracing; returns (output, BassKernelResults)."""
    nc = get_nc()
    in_maps = make_in_maps(x, Wq, Wk, Wv, Wo)
    res = run_bass_kernel_spmd(
        nc, in_maps, list(range(N_CORES)), trace=True, trace_cores=trace_cores
    )
    return _combine(res.results, bo), res


# revision 24
# speedup vs baseline: 1.1707x; 1.0714x over previous
"""Trainium2 Bass kernel for nn_MultiHeadAttention_55894704390646.

Multi-head causal attention, B=2, S=2048, E=1024, H=16 heads, D=64.
Sharding: data-parallel over batch (2 groups) x tensor-parallel over heads
(4 heads per core). Each core computes a partial output-projection result
(row-split Wo); the host sums the 4 partials per batch and adds the bias.

Design (all matmul operands bf16, f32 PSUM accumulation):
  - host supplies x^T [E, S] and pre-transposed weight slices in bf16, so
    every matmul contraction lands on SBUF partitions with no on-device
    transposes, and DMA bytes are halved. Output is stored bf16.
  - scores are computed transposed ([keys, queries]); softmax uses exp(s/8)
    (no max subtraction: |s/8| is bounded) and the denominator comes from a
    ones-column appended to v (AV lhsT free size 65).
  - causal narrowing: diagonal-region k-blocks only compute/exp/AV the live
    query range [128*i, 512); one [128,128] triangle mask zeroes the
    partial block via a single bf16 DVE multiply.
  - v / next-chunk q,k / prev-chunk output-projection matmuls are emitted
    as *fillers* between attention groups (rationed so they last the whole
    chunk, with a reserve for the tail) to keep the in-order PE queue dense
    and to hide exp latency and short-matmul LDWEIGHTS costs.
  - scores run two groups ahead of AV (prefetch depth 2 across pair
    boundaries) so AV never waits on ACT exp latency.
  - normalize: denominator row staged to SBUF (reciprocal cannot read PSUM),
    reciprocal on DVE, partition-broadcast on Pool, multiply on DVE; the
    head-1 half reaches SBUF partitions 64:128 via a Pool-queue SBUF-to-SBUF
    DMA, except for the program-final pair where a PE identity-matmul
    partition move avoids wedging the Tensor queue on a DMA semaphore.
  - engine balance: exp + q/k evac (+ late-chunk out evac) on ACT, masks /
    reciprocal / normalize / v / early out evac on DVE, broadcasts + weight
    loads + shift DMAs on Pool, x loads and stores on the sync queue.
"""

import sys

if "/opt/trn_rl_repo" not in sys.path:
    sys.path.insert(0, "/opt/trn_rl_repo")

import numpy as np
import ml_dtypes

import concourse.bass as bass
from concourse import bacc
import concourse.mybir as mybir
import concourse.tile as tile
from concourse.bass_utils import run_bass_kernel_spmd

B, S, E, H, D = 2, 2048, 1024, 16, 64
N_CORES = 8
DP = 2                 # batch groups
TP = 4                 # cores per batch group
HL = H // TP           # local heads per core = 4
DL = HL * D            # local head dims = 256
P = 128
NTB = S // P           # token blocks = 16
QC = 512               # query chunk
NQC = S // QC          # query chunks = 4
NKB = QC // P          # k-blocks per q chunk = 4
NPAIR = HL // 2        # head pairs = 2
NEO = E // QC          # output feature chunks of 512 = 2
NKO = E // P           # contraction blocks over E = 8

f32 = mybir.dt.float32
bf16 = mybir.dt.bfloat16
EXP = mybir.ActivationFunctionType.Exp

_NC_CACHE = None


def _build_nc():
    nc = bacc.Bacc("TRN2", target_bir_lowering=False, debug=False)

    xT = nc.dram_tensor("xT", (E, S), bf16, kind="ExternalInput")
    wqT = nc.dram_tensor("wqT", (E, DL), bf16, kind="ExternalInput")
    wkT = nc.dram_tensor("wkT", (E, DL), bf16, kind="ExternalInput")
    wvT = nc.dram_tensor("wvT", (E, DL), bf16, kind="ExternalInput")
    woT = nc.dram_tensor("woT", (DL, E), bf16, kind="ExternalInput")
    out = nc.dram_tensor("out", (S, E), bf16, kind="ExternalOutput")

    with tile.TileContext(nc) as tc:
        with (
            nc.allow_low_precision(reason="bf16 operands / f32 accumulation"),
            tc.tile_pool(name="big", bufs=1) as big,
            tc.tile_pool(name="work", bufs=5) as work,
            tc.tile_pool(name="w2", bufs=4) as w2,
            tc.tile_pool(name="ps_s", bufs=2, space="PSUM") as ps_s,
            tc.tile_pool(name="ps_ctx", bufs=2, space="PSUM") as ps_ctx,
            tc.tile_pool(name="ps_mm", bufs=2, space="PSUM") as ps_mm,
        ):
            # ---- input loads (sync queue), priority order ----------------
            # per-ko weight tiles + per-(ko, chunk) x tiles so the first
            # projection chain starts as soon as its first pieces land.
            wq_t = [big.tile([P, DL], bf16, tag=f"wq{ko}", name=f"wq{ko}") for ko in range(NKO)]
            wk_t = [big.tile([P, DL], bf16, tag=f"wk{ko}", name=f"wk{ko}") for ko in range(NKO)]
            wv_t = [big.tile([P, DL], bf16, tag=f"wv{ko}", name=f"wv{ko}") for ko in range(NKO)]
            xq = [
                [big.tile([P, QC], bf16, tag=f"x{ko}_{c}", name=f"x{ko}_{c}") for c in range(NQC)]
                for ko in range(NKO)
            ]
            wqT_r = wqT[:].rearrange("(ko p) d -> ko p d", p=P)
            wkT_r = wkT[:].rearrange("(ko p) d -> ko p d", p=P)
            wvT_r = wvT[:].rearrange("(ko p) d -> ko p d", p=P)
            xT_r = xT[:].rearrange("(ko p) (c s) -> ko p c s", p=P, c=NQC)
            for ko in range(NKO):
                nc.gpsimd.dma_start(wq_t[ko][:], wqT_r[ko, :, :])
                nc.sync.dma_start(xq[ko][0][:], xT_r[ko, :, 0, :])
            for ko in range(NKO):
                nc.gpsimd.dma_start(wk_t[ko][:], wkT_r[ko, :, :])
            for ko in range(NKO):
                nc.gpsimd.dma_start(wv_t[ko][:], wvT_r[ko, :, :])
            for ko in range(NKO):
                nc.sync.dma_start(xq[ko][1][:], xT_r[ko, :, 1, :])
            woT_sb = big.tile([P, NPAIR, E], bf16, tag="woT", name="woT")
            nc.gpsimd.dma_start(woT_sb[:], woT[:].rearrange("(pr p) e -> p pr e", p=P))
            for c in (2, 3):
                for ko in range(NKO):
                    nc.sync.dma_start(xq[ko][c][:], xT_r[ko, :, c, :])

            # ---- constants (Pool) ---------------------------------------
            # triangle mask: mask[k, q] = 1 if k <= q else 0
            mask = big.tile([P, P], bf16, tag="mask", name="mask")
            nc.gpsimd.memset(mask[:], 1.0)
            nc.gpsimd.affine_select(
                out=mask[:],
                in_=mask[:],
                compare_op=mybir.AluOpType.is_ge,
                fill=0.0,
                base=0,
                pattern=[[1, P]],
                channel_multiplier=-1,
            )

            # identity [64,64] for PE partition-moves (ctx1 -> rows 64:128)
            id64 = big.tile([64, 64], bf16, tag="id64", name="id64")
            nc.gpsimd.memset(id64[:], 1.0)
            nc.gpsimd.affine_select(
                out=id64[:], in_=id64[:], compare_op=mybir.AluOpType.is_equal,
                fill=0.0, base=0, pattern=[[1, 64]], channel_multiplier=-1,
            )

            # ---- persistent activation tiles ----------------------------
            qT_c = [[None] * NQC for _ in range(NPAIR)]
            kT_c = [[None] * NQC for _ in range(NPAIR)]
            for pr in range(NPAIR):
                for ch in range(NQC):
                    qT_c[pr][ch] = big.tile([P, QC], bf16, tag=f"qT{pr}{ch}", name=f"qT{pr}{ch}")
                    kT_c[pr][ch] = big.tile([P, QC], bf16, tag=f"kT{pr}{ch}", name=f"kT{pr}{ch}")
            v_tb = []
            for tb in range(NTB):
                vt = big.tile([P, HL, D + 1], bf16, tag=f"v{tb}", name=f"v{tb}")
                nc.gpsimd.memset(vt[:, :, D], 1.0)
                v_tb.append(vt)
            ctx_J = [
                big.tile([P, NPAIR, QC], bf16, tag=f"ctxT{J}", name=f"ctxT{J}") for J in range(NQC)
            ]

            # ---- filler generators (one PE matmul per yield) ------------
            def gen_qk(ch):
                for pr in range(NPAIR):
                    for wt, dst in ((wq_t, qT_c), (wk_t, kT_c)):
                        pp = ps_mm.tile([P, QC], f32, tag="mm", name=f"pqk{pr}{ch}")
                        for ko in range(NKO):
                            nc.tensor.matmul(
                                pp[:],
                                wt[ko][:, pr * P : (pr + 1) * P],
                                xq[ko][ch][:],
                                start=(ko == 0),
                                stop=(ko == NKO - 1),
                            )
                            yield
                        nc.scalar.copy(dst[pr][ch][:], pp[:])

            def gen_v(tb0, tb1):
                for tb in range(tb0, tb1):
                    pv_full = ps_mm.tile([P, QC], f32, tag="mm", name=f"pv{tb}")
                    pv = pv_full[:, 0:DL]
                    tsl = slice((tb % NKB) * P, (tb % NKB + 1) * P)
                    for ko in range(NKO):
                        nc.tensor.matmul(
                            pv[:],
                            xq[ko][tb // NKB][:, tsl],
                            wv_t[ko][:],
                            start=(ko == 0),
                            stop=(ko == NKO - 1),
                        )
                        yield
                    nc.vector.tensor_copy(
                        v_tb[tb][:, :, 0:D],
                        pv[:].rearrange("p (h d) -> p h d", h=HL),
                    )

            moves = {}

            def gen_out(J):
                # finalize ctx_J[J] rows 64:128: PE move + DVE evac, deferred
                # here so the move's input is long ready (no PE queue stall)
                for pr in range(NPAIR):
                    mv = moves.pop((pr, J), None)
                    if mv is not None:
                        mv()
                for tb in range(NKB * J, NKB * (J + 1)):
                    tsl = slice((tb % NKB) * P, (tb % NKB + 1) * P)
                    for ec in range(NEO):
                        o_ps = ps_mm.tile([P, QC], f32, tag="mm", name=f"o{tb}{ec}")
                        for pr in range(NPAIR):
                            nc.tensor.matmul(
                                o_ps[:],
                                ctx_J[J][:, pr, tsl],
                                woT_sb[:, pr, ec * QC : (ec + 1) * QC],
                                start=(pr == 0),
                                stop=(pr == NPAIR - 1),
                            )
                            yield
                        o_sb = w2.tile([P, QC], bf16, tag="osb", name=f"ob{tb}{ec}")
                        # last chunk: evacuate on ACT (idle there) so DVE
                        # stays clear for the tail normalize chain
                        if J >= 3:
                            nc.scalar.copy(o_sb[:], o_ps[:])
                        else:
                            nc.vector.tensor_copy(o_sb[:], o_ps[:])
                        st_eng = nc.gpsimd if (J == NQC - 1 and ec == 1) else nc.sync
                        st_eng.dma_start(
                            out[tb * P : (tb + 1) * P, ec * QC : (ec + 1) * QC],
                            o_sb[:],
                        )

            # filler queue: list of [kind, chunk, generator, remaining]
            fillers = []
            reserve = [0]  # keep >= this many filler matmuls for the tail
            out_budget = [0]  # per-group cap on "out"-kind pops, so the
            # output-projection fillers spread across the whole chunk instead
            # of draining in a burst and leaving later groups bare

            def remaining():
                return sum(f[3] for f in fillers)

            def drain(n):
                k = 0
                while fillers and k < n and remaining() > reserve[0]:
                    if fillers[0][0] == "out":
                        if out_budget[0] <= 0:
                            break
                        out_budget[0] -= 1
                    try:
                        next(fillers[0][2])
                        fillers[0][3] -= 1
                        k += 1
                    except StopIteration:
                        fillers.pop(0)

            def flush(kind, upto):
                i = 0
                while i < len(fillers):
                    if fillers[i][0] == kind and fillers[i][1] <= upto:
                        for _ in fillers[i][2]:
                            pass
                        fillers.pop(i)
                    else:
                        i += 1

            def flush_all():
                while fillers:
                    for _ in fillers[0][2]:
                        pass
                    fillers.pop(0)

            # ---- attention ---------------------------------------------
            def scores_group(pr, J, I):
                """Scores + exp (+ triangle mask) for k-block I of (pr, J).

                Returns (pT, qoff): exp'd probabilities, transposed
                [keys, head, queries], valid for columns [qoff:QC)."""
                kch, ib = divmod(I, NKB)
                ik = slice(ib * P, (ib + 1) * P)
                di = I - NKB * J
                qoff = di * P if di >= 0 else 0
                s = ps_s.tile([P, 2, QC], f32, tag="s", name=f"s{pr}{J}{I}")
                nc.tensor.matmul(
                    s[:, 0, qoff:],
                    kT_c[pr][kch][0:64, ik],
                    qT_c[pr][J][0:64, qoff:],
                    start=True,
                    stop=True,
                )
                nc.tensor.matmul(
                    s[:, 1, qoff:],
                    kT_c[pr][kch][64:128, ik],
                    qT_c[pr][J][64:128, qoff:],
                    start=True,
                    stop=True,
                )
                pT = work.tile([P, 2, QC], bf16, tag="pT", name=f"pT{pr}{J}{I}")
                nc.scalar.activation(pT[:, :, qoff:], s[:, :, qoff:], EXP, scale=0.125)
                if di >= 0:
                    nc.vector.tensor_tensor(
                        pT[:, :, qoff : qoff + P],
                        pT[:, :, qoff : qoff + P],
                        mask[:, None, :].to_broadcast((P, 2, P)),
                        mybir.AluOpType.mult,
                    )
                return pT, qoff

            def normalize(ctx_ps, pr, r, J):
                """ctx_J[h-half] = ctx[0:64] / ctx[64] (bf16).

                r=1 goes through an SBUF shift DMA into partitions 64:128."""
                # reciprocal_approx_fast reads garbage from PSUM (measured on
                # HW) — stage the denominator row through SBUF first.
                dn = w2.tile([1, QC], f32, tag="dn", name=f"dn{pr}{r}{J}")
                nc.vector.tensor_copy(dn[:], ctx_ps[D : D + 1, :])
                rc = w2.tile([1, QC], f32, tag="rc", name=f"rc{pr}{r}{J}")
                nc.vector.reciprocal_approx_fast(rc[:], dn[:])
                rb = w2.tile([64, QC], f32, tag="rb", name=f"rb{pr}{r}{J}")
                nc.gpsimd.partition_broadcast(rb[:], rc[:])
                if r == 0:
                    nc.vector.tensor_tensor(
                        ctx_J[J][0:64, pr, :],
                        ctx_ps[0:D, :],
                        rb[:],
                        mybir.AluOpType.mult,
                    )
                else:
                    tmp = w2.tile([64, QC], bf16, tag="tmp", name=f"ct{pr}{J}")
                    nc.vector.tensor_tensor(
                        tmp[:], ctx_ps[0:D, :], rb[:], mybir.AluOpType.mult
                    )
                    if (pr, J) == (1, NQC - 1):
                        # program-final pair: a shift DMA here wedges the
                        # in-order Tensor queue at the tail (and drops the PE
                        # pstate); use a PE partition-move instead, deferred
                        # until after the reserved tail fillers.
                        def mv(tmp=tmp, pr=pr, J=J):
                            pm = ps_mm.tile([P, QC], f32, tag="mm", name="mvz")
                            nc.tensor.matmul(
                                pm[64:128, :], id64[:], tmp[:],
                                start=True, stop=True,
                            )
                            nc.vector.tensor_copy(
                                ctx_J[J][64:128, pr, :], pm[64:128, :]
                            )

                        moves[(pr, J)] = mv
                    else:
                        nc.gpsimd.dma_start(ctx_J[J][64:128, pr, :], tmp[:])

            def emit_attn_pair(pr, J, tail=False):
                """Attention for head pair (2pr, 2pr+1) on query chunk J.

                AVs are skewed one k-block behind the scores so the in-order
                PE queue never waits on exp/mask; fillers emitted between
                groups keep the PE dense and deepen the skew."""
                h0, h1 = 2 * pr, 2 * pr + 1
                order = list(range(NKB * (J + 1)))
                ctx0 = ps_ctx.tile([D + 1, QC], f32, tag="ctx", name=f"c0_{pr}{J}")
                ctx1 = ps_ctx.tile([D + 1, QC], f32, tag="ctx", name=f"c1_{pr}{J}")

                def emit_av(pos, I, pT, qoff, mid_drain=0):
                    first, last = pos == 0, pos == len(order) - 1
                    nc.tensor.matmul(
                        ctx0[:, qoff:], v_tb[I][:, h0, :], pT[:, 0, qoff:],
                        start=first, stop=last,
                    )
                    if mid_drain:
                        drain(mid_drain)
                    nc.tensor.matmul(
                        ctx1[:, qoff:], v_tb[I][:, h1, :], pT[:, 1, qoff:],
                        start=first, stop=last,
                    )

                if not pending:
                    pending.append(scores_group(pr, J, order[0]))
                    pending.append(scores_group(pr, J, order[1]))
                for pos in range(2, len(order)):
                    out_budget[0] = 1
                    pending.append(scores_group(pr, J, order[pos]))
                    drain(3)
                    emit_av(pos - 2, order[pos - 2], *pending.pop(0), mid_drain=1)
                    drain(3)
                # prefetch the NEXT pair's first two scores groups before the
                # last AVs + normalize so the PE queue never drains at pair
                # starts and AV never waits on exp latency
                nxt = chain.pop(0) if chain else None
                nxt_pending = []
                if nxt is not None:
                    if nxt[1] != J:
                        flush("qk", nxt[1])
                    nxt_pending.append(scores_group(nxt[0], nxt[1], 0))
                emit_av(len(order) - 2, order[-2], *pending.pop(0), mid_drain=1)
                if nxt is not None:
                    nxt_pending.append(scores_group(nxt[0], nxt[1], 1))
                emit_av(len(order) - 1, order[-1], *pending.pop(0))
                pending.extend(nxt_pending)
                normalize(ctx1, pr, 1, J)
                if tail:
                    deferred_ctx0.append((ctx0, pr, J))
                else:
                    normalize(ctx0, pr, 0, J)

            # ---- schedule ----------------------------------------------
            chain = [(0, 0), (1, 0), (0, 1), (1, 1), (0, 2), (1, 2), (0, 3), (1, 3)]
            pending = []
            chain.pop(0)

            for _ in gen_qk(0):
                pass
            for _ in gen_v(0, NKB):
                pass
            fillers.append(["qk", 1, gen_qk(1), 32])
            fillers.append(["v", 1, gen_v(NKB, 2 * NKB), 32])
            deferred_ctx0 = []
            for J in range(NQC):
                # during the last chunk, hold back fillers so the PE has work
                # while the final normalize/move chain completes
                reserve[0] = 18 if J == NQC - 1 else 0
                flush("v", J)
                emit_attn_pair(0, J)
                emit_attn_pair(1, J, tail=(J == NQC - 1))
                if J + 2 <= NQC - 1:
                    fillers.append(["qk", J + 2, gen_qk(J + 2), 32])
                    fillers.append(
                        ["v", J + 2, gen_v(NKB * (J + 2), NKB * (J + 3)), 32]
                    )
                if J < NQC - 1:
                    fillers.append(["out", J, gen_out(J), 16])
            reserve[0] = 0
            flush_all()
            for c0, pr0, J0 in deferred_ctx0:
                normalize(c0, pr0, 0, J0)
            mv = moves.pop((1, NQC - 1), None)
            if mv is not None:
                mv()
            for _ in gen_out(NQC - 1):
                pass

    nc.compile()
    return nc


def get_nc():
    global _NC_CACHE
    if _NC_CACHE is None:
        _NC_CACHE = _build_nc()
    return _NC_CACHE


def make_in_maps(x, Wq, Wk, Wv, Wo):
    bf = ml_dtypes.bfloat16
    x = np.asarray(x, dtype=np.float32)
    Wq = np.asarray(Wq, dtype=np.float32)
    Wk = np.asarray(Wk, dtype=np.float32)
    Wv = np.asarray(Wv, dtype=np.float32)
    Wo = np.asarray(Wo, dtype=np.float32)
    in_maps = []
    for c in range(N_CORES):
        b, g = divmod(c, TP)
        sl = slice(DL * g, DL * (g + 1))
        in_maps.append(
            {
                "xT": np.ascontiguousarray(x[b].T).astype(bf),
                "wqT": np.ascontiguousarray(Wq[sl].T).astype(bf),
                "wkT": np.ascontiguousarray(Wk[sl].T).astype(bf),
                "wvT": np.ascontiguousarray(Wv[sl].T).astype(bf),
                "woT": np.ascontiguousarray(Wo[:, sl].T).astype(bf),
            }
        )
    return in_maps


def _combine(results, bo):
    bo = np.asarray(bo, dtype=np.float32)
    y = np.zeros((B, S, E), dtype=np.float32)
    for c in range(N_CORES):
        y[c // TP] += results[c]["out"].astype(np.float32)
    y += bo
    return y


def kernel(x, Wq, Wk, Wv, Wo, bo):
    nc = get_nc()
    in_maps = make_in_maps(x, Wq, Wk, Wv, Wo)
    res = run_bass_kernel_spmd(nc, in_maps, list(range(N_CORES)))
    return _combine(res.results, bo)


def kernel_traced(x, Wq, Wk, Wv, Wo, bo, trace_cores=None):
    """Like kernel() but with NTFF tracing; returns (output, BassKernelResults)."""
    nc = get_nc()
    in_maps = make_in_maps(x, Wq, Wk, Wv, Wo)
    res = run_bass_kernel_spmd(
        nc, in_maps, list(range(N_CORES)), trace=True, trace_cores=trace_cores
    )
    return _combine(res.results, bo), res


# revision 25
# speedup vs baseline: 1.1894x; 1.0160x over previous
"""Trainium2 Bass kernel for nn_MultiHeadAttention_55894704390646.

Multi-head causal attention, B=2, S=2048, E=1024, H=16 heads, D=64.
Sharding: data-parallel over batch (2 groups) x tensor-parallel over heads
(4 heads per core). Each core computes a partial output-projection result
(row-split Wo); the host sums the 4 partials per batch and adds the bias.

Design (all matmul operands bf16, f32 PSUM accumulation):
  - host supplies x^T [E, S] and pre-transposed weight slices in bf16, so
    every matmul contraction lands on SBUF partitions with no on-device
    transposes, and DMA bytes are halved. Output is stored bf16.
  - scores are computed transposed ([keys, queries]); softmax uses exp(s/8)
    (no max subtraction: |s/8| is bounded) and the denominator comes from a
    ones-column appended to v (AV lhsT free size 65).
  - causal narrowing: diagonal-region k-blocks only compute/exp/AV the live
    query range [128*i, 512); one [128,128] triangle mask zeroes the
    partial block via a single bf16 DVE multiply.
  - v / next-chunk q,k / prev-chunk output-projection matmuls are emitted
    as *fillers* between attention groups (rationed so they last the whole
    chunk, with a reserve for the tail) to keep the in-order PE queue dense
    and to hide exp latency and short-matmul LDWEIGHTS costs.
  - scores run two groups ahead of AV (prefetch depth 2 across pair
    boundaries) so AV never waits on ACT exp latency.
  - normalize: denominator row staged to SBUF (reciprocal cannot read PSUM),
    reciprocal on DVE, partition-broadcast on Pool, multiply on DVE; the
    head-1 half reaches SBUF partitions 64:128 via a Pool-queue SBUF-to-SBUF
    DMA, except for the program-final pair where a PE identity-matmul
    partition move avoids wedging the Tensor queue on a DMA semaphore.
  - engine balance: exp + q/k evac (+ late-chunk out evac) on ACT, masks /
    reciprocal / normalize / v / early out evac on DVE, broadcasts + weight
    loads + shift DMAs on Pool, x loads and stores on the sync queue.
"""

import sys

if "/opt/trn_rl_repo" not in sys.path:
    sys.path.insert(0, "/opt/trn_rl_repo")

import numpy as np
import ml_dtypes

import concourse.bass as bass
from concourse import bacc
import concourse.mybir as mybir
import concourse.tile as tile
from concourse.bass_utils import run_bass_kernel_spmd

B, S, E, H, D = 2, 2048, 1024, 16, 64
N_CORES = 8
DP = 2                 # batch groups
TP = 4                 # cores per batch group
HL = H // TP           # local heads per core = 4
DL = HL * D            # local head dims = 256
P = 128
NTB = S // P           # token blocks = 16
QC = 512               # query chunk
NQC = S // QC          # query chunks = 4
NKB = QC // P          # k-blocks per q chunk = 4
NPAIR = HL // 2        # head pairs = 2
NEO = E // QC          # output feature chunks of 512 = 2
NKO = E // P           # contraction blocks over E = 8

f32 = mybir.dt.float32
bf16 = mybir.dt.bfloat16
EXP = mybir.ActivationFunctionType.Exp

_NC_CACHE = None


def _build_nc():
    nc = bacc.Bacc("TRN2", target_bir_lowering=False, debug=False)

    xT = nc.dram_tensor("xT", (E, S), bf16, kind="ExternalInput")
    wqT = nc.dram_tensor("wqT", (E, DL), bf16, kind="ExternalInput")
    wkT = nc.dram_tensor("wkT", (E, DL), bf16, kind="ExternalInput")
    wvT = nc.dram_tensor("wvT", (E, DL), bf16, kind="ExternalInput")
    woT = nc.dram_tensor("woT", (DL, E), bf16, kind="ExternalInput")
    out = nc.dram_tensor("out", (S, E), bf16, kind="ExternalOutput")

    with tile.TileContext(nc) as tc:
        with (
            nc.allow_low_precision(reason="bf16 operands / f32 accumulation"),
            tc.tile_pool(name="big", bufs=1) as big,
            tc.tile_pool(name="work", bufs=5) as work,
            tc.tile_pool(name="w2", bufs=4) as w2,
            tc.tile_pool(name="ps_s", bufs=2, space="PSUM") as ps_s,
            tc.tile_pool(name="ps_ctx", bufs=2, space="PSUM") as ps_ctx,
            tc.tile_pool(name="ps_mm", bufs=2, space="PSUM") as ps_mm,
        ):
            # ---- input loads (sync queue), priority order ----------------
            # per-ko weight tiles + per-(ko, chunk) x tiles so the first
            # projection chain starts as soon as its first pieces land.
            wq_t = [big.tile([P, DL], bf16, tag=f"wq{ko}", name=f"wq{ko}") for ko in range(NKO)]
            wk_t = [big.tile([P, DL], bf16, tag=f"wk{ko}", name=f"wk{ko}") for ko in range(NKO)]
            wv_t = [big.tile([P, DL], bf16, tag=f"wv{ko}", name=f"wv{ko}") for ko in range(NKO)]
            xq = [
                [big.tile([P, QC], bf16, tag=f"x{ko}_{c}", name=f"x{ko}_{c}") for c in range(NQC)]
                for ko in range(NKO)
            ]
            wqT_r = wqT[:].rearrange("(ko p) d -> ko p d", p=P)
            wkT_r = wkT[:].rearrange("(ko p) d -> ko p d", p=P)
            wvT_r = wvT[:].rearrange("(ko p) d -> ko p d", p=P)
            xT_r = xT[:].rearrange("(ko p) (c s) -> ko p c s", p=P, c=NQC)
            for ko in range(NKO):
                nc.gpsimd.dma_start(wq_t[ko][:], wqT_r[ko, :, :])
                nc.sync.dma_start(xq[ko][0][:], xT_r[ko, :, 0, :])
            for ko in range(NKO):
                nc.gpsimd.dma_start(wk_t[ko][:], wkT_r[ko, :, :])
            for ko in range(NKO):
                nc.gpsimd.dma_start(wv_t[ko][:], wvT_r[ko, :, :])
            for ko in range(NKO):
                nc.sync.dma_start(xq[ko][1][:], xT_r[ko, :, 1, :])
            woT_sb = big.tile([P, NPAIR, E], bf16, tag="woT", name="woT")
            nc.gpsimd.dma_start(woT_sb[:], woT[:].rearrange("(pr p) e -> p pr e", p=P))
            for c in (2, 3):
                for ko in range(NKO):
                    nc.sync.dma_start(xq[ko][c][:], xT_r[ko, :, c, :])

            # ---- constants (Pool) ---------------------------------------
            # triangle mask: mask[k, q] = 1 if k <= q else 0
            mask = big.tile([P, P], bf16, tag="mask", name="mask")
            nc.gpsimd.memset(mask[:], 1.0)
            nc.gpsimd.affine_select(
                out=mask[:],
                in_=mask[:],
                compare_op=mybir.AluOpType.is_ge,
                fill=0.0,
                base=0,
                pattern=[[1, P]],
                channel_multiplier=-1,
            )

            # identity [64,64] for PE partition-moves (ctx1 -> rows 64:128)
            id64 = big.tile([64, 64], bf16, tag="id64", name="id64")
            nc.gpsimd.memset(id64[:], 1.0)
            nc.gpsimd.affine_select(
                out=id64[:], in_=id64[:], compare_op=mybir.AluOpType.is_equal,
                fill=0.0, base=0, pattern=[[1, 64]], channel_multiplier=-1,
            )

            # ---- persistent activation tiles ----------------------------
            qT_c = [[None] * NQC for _ in range(NPAIR)]
            kT_c = [[None] * NQC for _ in range(NPAIR)]
            for pr in range(NPAIR):
                for ch in range(NQC):
                    qT_c[pr][ch] = big.tile([P, QC], bf16, tag=f"qT{pr}{ch}", name=f"qT{pr}{ch}")
                    kT_c[pr][ch] = big.tile([P, QC], bf16, tag=f"kT{pr}{ch}", name=f"kT{pr}{ch}")
            v_tb = []
            for tb in range(NTB):
                vt = big.tile([P, HL, D + 1], bf16, tag=f"v{tb}", name=f"v{tb}")
                nc.gpsimd.memset(vt[:, :, D], 1.0)
                v_tb.append(vt)
            ctx_J = [
                big.tile([P, NPAIR, QC], bf16, tag=f"ctxT{J}", name=f"ctxT{J}") for J in range(NQC)
            ]

            # ---- filler generators (one PE matmul per yield) ------------
            def gen_qk(ch):
                for pr in range(NPAIR):
                    for wt, dst in ((wq_t, qT_c), (wk_t, kT_c)):
                        pp = ps_mm.tile([P, QC], f32, tag="mm", name=f"pqk{pr}{ch}")
                        for ko in range(NKO):
                            nc.tensor.matmul(
                                pp[:],
                                wt[ko][:, pr * P : (pr + 1) * P],
                                xq[ko][ch][:],
                                start=(ko == 0),
                                stop=(ko == NKO - 1),
                            )
                            yield
                        nc.scalar.copy(dst[pr][ch][:], pp[:])

            def gen_v(tb0, tb1):
                for tb in range(tb0, tb1):
                    pv_full = ps_mm.tile([P, QC], f32, tag="mm", name=f"pv{tb}")
                    pv = pv_full[:, 0:DL]
                    tsl = slice((tb % NKB) * P, (tb % NKB + 1) * P)
                    for ko in range(NKO):
                        nc.tensor.matmul(
                            pv[:],
                            xq[ko][tb // NKB][:, tsl],
                            wv_t[ko][:],
                            start=(ko == 0),
                            stop=(ko == NKO - 1),
                        )
                        yield
                    nc.vector.tensor_copy(
                        v_tb[tb][:, :, 0:D],
                        pv[:].rearrange("p (h d) -> p h d", h=HL),
                    )

            moves = {}

            def gen_out(J):
                # finalize ctx_J[J] rows 64:128: PE move + DVE evac, deferred
                # here so the move's input is long ready (no PE queue stall)
                for pr in range(NPAIR):
                    mv = moves.pop((pr, J), None)
                    if mv is not None:
                        mv()
                for tb in range(NKB * J, NKB * (J + 1)):
                    tsl = slice((tb % NKB) * P, (tb % NKB + 1) * P)
                    for ec in range(NEO):
                        o_ps = ps_mm.tile([P, QC], f32, tag="mm", name=f"o{tb}{ec}")
                        for pr in range(NPAIR):
                            nc.tensor.matmul(
                                o_ps[:],
                                ctx_J[J][:, pr, tsl],
                                woT_sb[:, pr, ec * QC : (ec + 1) * QC],
                                start=(pr == 0),
                                stop=(pr == NPAIR - 1),
                            )
                            yield
                        o_sb = w2.tile([P, QC], bf16, tag="osb", name=f"ob{tb}{ec}")
                        # late chunks: evacuate on ACT (idle there) so DVE
                        # stays clear for the tail normalize chain
                        if J >= 2:
                            nc.scalar.copy(o_sb[:], o_ps[:])
                        else:
                            nc.vector.tensor_copy(o_sb[:], o_ps[:])
                        nc.sync.dma_start(
                            out[tb * P : (tb + 1) * P, ec * QC : (ec + 1) * QC],
                            o_sb[:],
                        )

            # filler queue: list of [kind, chunk, generator, remaining]
            fillers = []
            reserve = [0]  # keep >= this many filler matmuls for the tail
            out_budget = [0]  # per-group cap on "out"-kind pops, so the
            # output-projection fillers spread across the whole chunk instead
            # of draining in a burst and leaving later groups bare

            def remaining():
                return sum(f[3] for f in fillers)

            def drain(n):
                k = 0
                while fillers and k < n and remaining() > reserve[0]:
                    if fillers[0][0] == "out":
                        if out_budget[0] <= 0:
                            break
                        out_budget[0] -= 1
                    try:
                        next(fillers[0][2])
                        fillers[0][3] -= 1
                        k += 1
                    except StopIteration:
                        fillers.pop(0)

            def flush(kind, upto):
                i = 0
                while i < len(fillers):
                    if fillers[i][0] == kind and fillers[i][1] <= upto:
                        for _ in fillers[i][2]:
                            pass
                        fillers.pop(i)
                    else:
                        i += 1

            def flush_all():
                while fillers:
                    for _ in fillers[0][2]:
                        pass
                    fillers.pop(0)

            # ---- attention ---------------------------------------------
            def scores_group(pr, J, I):
                """Scores + exp (+ triangle mask) for k-block I of (pr, J).

                Returns (pT, qoff): exp'd probabilities, transposed
                [keys, head, queries], valid for columns [qoff:QC)."""
                kch, ib = divmod(I, NKB)
                ik = slice(ib * P, (ib + 1) * P)
                di = I - NKB * J
                qoff = di * P if di >= 0 else 0
                s = ps_s.tile([P, 2, QC], f32, tag="s", name=f"s{pr}{J}{I}")
                nc.tensor.matmul(
                    s[:, 0, qoff:],
                    kT_c[pr][kch][0:64, ik],
                    qT_c[pr][J][0:64, qoff:],
                    start=True,
                    stop=True,
                )
                nc.tensor.matmul(
                    s[:, 1, qoff:],
                    kT_c[pr][kch][64:128, ik],
                    qT_c[pr][J][64:128, qoff:],
                    start=True,
                    stop=True,
                )
                pT = work.tile([P, 2, QC], bf16, tag="pT", name=f"pT{pr}{J}{I}")
                nc.scalar.activation(pT[:, :, qoff:], s[:, :, qoff:], EXP, scale=0.125)
                if di >= 0:
                    nc.vector.tensor_tensor(
                        pT[:, :, qoff : qoff + P],
                        pT[:, :, qoff : qoff + P],
                        mask[:, None, :].to_broadcast((P, 2, P)),
                        mybir.AluOpType.mult,
                    )
                return pT, qoff

            def normalize(ctx_ps, pr, r, J):
                """ctx_J[h-half] = ctx[0:64] / ctx[64] (bf16).

                r=1 goes through an SBUF shift DMA into partitions 64:128."""
                # reciprocal_approx_fast reads garbage from PSUM (measured on
                # HW) — stage the denominator row through SBUF first.
                dn = w2.tile([1, QC], f32, tag="dn", name=f"dn{pr}{r}{J}")
                nc.vector.tensor_copy(dn[:], ctx_ps[D : D + 1, :])
                rc = w2.tile([1, QC], f32, tag="rc", name=f"rc{pr}{r}{J}")
                nc.vector.reciprocal_approx_fast(rc[:], dn[:])
                rb = w2.tile([64, QC], f32, tag="rb", name=f"rb{pr}{r}{J}")
                nc.gpsimd.partition_broadcast(rb[:], rc[:])
                if r == 0:
                    nc.vector.tensor_tensor(
                        ctx_J[J][0:64, pr, :],
                        ctx_ps[0:D, :],
                        rb[:],
                        mybir.AluOpType.mult,
                    )
                else:
                    tmp = w2.tile([64, QC], bf16, tag="tmp", name=f"ct{pr}{J}")
                    nc.vector.tensor_tensor(
                        tmp[:], ctx_ps[0:D, :], rb[:], mybir.AluOpType.mult
                    )
                    if (pr, J) == (1, NQC - 1):
                        # program-final pair: a shift DMA here wedges the
                        # in-order Tensor queue at the tail (and drops the PE
                        # pstate); use a PE partition-move instead, deferred
                        # until after the reserved tail fillers.
                        def mv(tmp=tmp, pr=pr, J=J):
                            pm = ps_mm.tile([P, QC], f32, tag="mm", name="mvz")
                            nc.tensor.matmul(
                                pm[64:128, :], id64[:], tmp[:],
                                start=True, stop=True,
                            )
                            nc.vector.tensor_copy(
                                ctx_J[J][64:128, pr, :], pm[64:128, :]
                            )

                        moves[(pr, J)] = mv
                    else:
                        nc.gpsimd.dma_start(ctx_J[J][64:128, pr, :], tmp[:])

            def emit_attn_pair(pr, J, tail=False):
                """Attention for head pair (2pr, 2pr+1) on query chunk J.

                AVs are skewed one k-block behind the scores so the in-order
                PE queue never waits on exp/mask; fillers emitted between
                groups keep the PE dense and deepen the skew."""
                h0, h1 = 2 * pr, 2 * pr + 1
                order = list(range(NKB * (J + 1)))
                ctx0 = ps_ctx.tile([D + 1, QC], f32, tag="ctx", name=f"c0_{pr}{J}")
                ctx1 = ps_ctx.tile([D + 1, QC], f32, tag="ctx", name=f"c1_{pr}{J}")

                def emit_av(pos, I, pT, qoff, mid_drain=0):
                    first, last = pos == 0, pos == len(order) - 1
                    nc.tensor.matmul(
                        ctx0[:, qoff:], v_tb[I][:, h0, :], pT[:, 0, qoff:],
                        start=first, stop=last,
                    )
                    if mid_drain:
                        drain(mid_drain)
                    nc.tensor.matmul(
                        ctx1[:, qoff:], v_tb[I][:, h1, :], pT[:, 1, qoff:],
                        start=first, stop=last,
                    )

                if not pending:
                    pending.append(scores_group(pr, J, order[0]))
                    pending.append(scores_group(pr, J, order[1]))
                for pos in range(2, len(order)):
                    out_budget[0] = 1
                    pending.append(scores_group(pr, J, order[pos]))
                    drain(3)
                    emit_av(pos - 2, order[pos - 2], *pending.pop(0), mid_drain=1)
                    drain(3)
                # prefetch the NEXT pair's first two scores groups before the
                # last AVs + normalize so the PE queue never drains at pair
                # starts and AV never waits on exp latency
                nxt = chain.pop(0) if chain else None
                nxt_pending = []
                if nxt is not None:
                    if nxt[1] != J:
                        flush("qk", nxt[1])
                    nxt_pending.append(scores_group(nxt[0], nxt[1], 0))
                emit_av(len(order) - 2, order[-2], *pending.pop(0), mid_drain=1)
                if nxt is not None:
                    nxt_pending.append(scores_group(nxt[0], nxt[1], 1))
                emit_av(len(order) - 1, order[-1], *pending.pop(0))
                pending.extend(nxt_pending)
                normalize(ctx1, pr, 1, J)
                if tail:
                    deferred_ctx0.append((ctx0, pr, J))
                else:
                    normalize(ctx0, pr, 0, J)

            # ---- schedule ----------------------------------------------
            chain = [(0, 0), (1, 0), (0, 1), (1, 1), (0, 2), (1, 2), (0, 3), (1, 3)]
            pending = []
            chain.pop(0)

            for _ in gen_qk(0):
                pass
            for _ in gen_v(0, NKB):
                pass
            fillers.append(["qk", 1, gen_qk(1), 32])
            fillers.append(["v", 1, gen_v(NKB, 2 * NKB), 32])
            deferred_ctx0 = []
            for J in range(NQC):
                # during the last chunk, hold back fillers so the PE has work
                # while the final normalize/move chain completes
                reserve[0] = 14 if J == NQC - 1 else 0
                flush("v", J)
                emit_attn_pair(0, J)
                emit_attn_pair(1, J, tail=(J == NQC - 1))
                if J + 2 <= NQC - 1:
                    fillers.append(["qk", J + 2, gen_qk(J + 2), 32])
                    fillers.append(
                        ["v", J + 2, gen_v(NKB * (J + 2), NKB * (J + 3)), 32]
                    )
                if J < NQC - 1:
                    fillers.append(["out", J, gen_out(J), 16])
            reserve[0] = 0
            flush_all()
            for c0, pr0, J0 in deferred_ctx0:
                normalize(c0, pr0, 0, J0)
            mv = moves.pop((1, NQC - 1), None)
            if mv is not None:
                mv()
            for _ in gen_out(NQC - 1):
                pass

    nc.compile()
    return nc


def get_nc():
    global _NC_CACHE
    if _NC_CACHE is None:
        _NC_CACHE = _build_nc()
    return _NC_CACHE


def make_in_maps(x, Wq, Wk, Wv, Wo):
    bf = ml_dtypes.bfloat16
    x = np.asarray(x, dtype=np.float32)
    Wq = np.asarray(Wq, dtype=np.float32)
    Wk = np.asarray(Wk, dtype=np.float32)
    Wv = np.asarray(Wv, dtype=np.float32)
    Wo = np.asarray(Wo, dtype=np.float32)
    in_maps = []
    for c in range(N_CORES):
        b, g = divmod(c, TP)
        sl = slice(DL * g, DL * (g + 1))
        in_maps.append(
            {
                "xT": np.ascontiguousarray(x[b].T).astype(bf),
                "wqT": np.ascontiguousarray(Wq[sl].T).astype(bf),
                "wkT": np.ascontiguousarray(Wk[sl].T).astype(bf),
                "wvT": np.ascontiguousarray(Wv[sl].T).astype(bf),
                "woT": np.ascontiguousarray(Wo[:, sl].T).astype(bf),
            }
        )
    return in_maps


def _combine(results, bo):
    bo = np.asarray(bo, dtype=np.float32)
    y = np.zeros((B, S, E), dtype=np.float32)
    for c in range(N_CORES):
        y[c // TP] += results[c]["out"].astype(np.float32)
    y += bo
    return y


def kernel(x, Wq, Wk, Wv, Wo, bo):
    nc = get_nc()
    in_maps = make_in_maps(x, Wq, Wk, Wv, Wo)
    res = run_bass_kernel_spmd(nc, in_maps, list(range(N_CORES)))
    return _combine(res.results, bo)


def kernel_traced(x, Wq, Wk, Wv, Wo, bo, trace_cores=None):
    """Like kernel() but with NTFF tracing; returns (output, BassKernelResults)."""
    nc = get_nc()
    in_maps = make_in_maps(x, Wq, Wk, Wv, Wo)
    res = run_bass_kernel_spmd(
        nc, in_maps, list(range(N_CORES)), trace=True, trace_cores=trace_cores
    )
    return _combine(res.results, bo), res
